# revision 1
# baseline (speedup 1.0000x reference)
"""Distributed TRN2 Bass kernel for AdaptiveGraphTopology pairwise edge MLP.

reference:
    a = emb @ W1a.T ; b = emb @ W1b.T           (W1a, W1b = W1[:, :H], W1[:, H:])
    hidden = relu(a[:,None,:] + b[None,:,:] + b1)      # [N,N,H]
    scores = hidden . W2[0] + b2                       # [N,N]
    weights = sigmoid(scores), zeroed diag
    mask    = (weights > 0.5) & ~eye

Sharding: rows i split across 8 cores (128 rows each); everything else
replicated. No collectives: each core DMAs out its row block, host
concatenates.

Per-core compute (mode "full" = v4, software-pipelined):
    BT[h, j] = b_j[h]        (all j)    -- f32r matmul on device
    CT[h, i] = a_i[h]+b1[h]  (local i)  -- f32r matmul + bias on device
    loop over local i:
      X_i[h, j] = relu(BT[h, j] + CT[h, i])   (DVE/ACT, fp32r out)
      scores[i, :] += w2 . X_i  via fp32r matmul whose stationary is a
      sliding window over Z[128, 256] (w2 at column 128, zeros elsewhere):
      window [128-i : 256-i] places w2 in PE column i, so row i's scores
      land in PSUM partition i and the 128 iterations accumulate a full
      [128, 1024] score block (zero columns contribute exact zeros).
    The diagonal is pushed to -1e30 by pre-initializing the score PSUM
    with accumulating matmuls (stationary -1e30*I, moving a per-core
    one-hot eye_rows matrix), so the epilogue is just:
    weights = sigmoid(scores+b2) (diag -> 0.0), mask = scores > -b2.

Measured facts (this device) driving the structure:
  - The PE streams moving data at ~2.0 cols/ns regardless of dtype
    (f32r/bf16), stationary width, PSUM bank pattern or MM size: the
    256 x 512-col score matmuls are a hard ~66 us floor; everything
    else must hide behind it.
  - tc.For_i puts an all-engine barrier at each iteration end, so
    cross-rep pipelining is impossible. BT/CT/eye-init are
    rep-invariant, so each body computes them for the NEXT rep during
    its epilogue/DMA tail (PE idle there), and the prologue seeds the
    first rep: the body then starts generating X_0 immediately.
"""
import numpy as np

N = 1024
H = 128
NCORES = 8
ROWS = N // NCORES  # 128 rows per core

_cache = {}


def _split_multiwaits(nc, limit=1):
    """This walrus build accepts only ONE semaphore wait/update per
    instruction; Tile emits several. Split extras onto adjacent NoOps."""
    import bass_rust

    f = nc.m.functions[0]
    engines = nc.engines

    def make_nop(engine_type):
        eng = engines[engine_type]
        inst = eng.nop(nofuse=True).ins
        for b in f.blocks:
            lst = b.instructions
            for k in range(len(lst) - 1, -1, -1):
                if lst[k] is inst:
                    lst.pop(k)
                    return inst
        return inst

    n_split = 0
    for b in f.blocks:
        insts = b.instructions
        i = 0
        while i < len(insts):
            inst = insts[i]
            si = inst.sync_info
            if si is None:
                i += 1
                continue
            waits = list(si.on_wait)
            ups = list(si.on_update)
            same_sem = (
                len(waits) >= 1 and len(ups) >= 1
                and any(getattr(w, "id", None) == getattr(u, "id", None)
                        for w in waits for u in ups)
            )
            if len(waits) <= limit and len(ups) <= 1 and not same_sem:
                i += 1
                continue
            pre = []
            post = []
            if len(waits) > limit:
                extra, waits = waits[: len(waits) - limit], waits[len(waits) - limit :]
                for w in extra:
                    nop = make_nop(inst.engine)
                    nop.sync_info = bass_rust.SyncInfo(on_wait=[w], on_update=[])
                    pre.append(nop)
            if len(ups) > 1:
                ups, extra_u = ups[:1], ups[1:]
                for u in extra_u:
                    nop = make_nop(inst.engine)
                    nop.sync_info = bass_rust.SyncInfo(on_wait=[], on_update=[u])
                    post.append(nop)
            if (waits and ups
                    and getattr(waits[0], "id", None) == getattr(ups[0], "id", None)):
                # wait+update on one semaphore in a single instruction trips
                # walrus's no_semaphore_value_conflict: hoist the wait onto
                # a preceding NoOp (engine queues are in-order)
                nop = make_nop(inst.engine)
                nop.sync_info = bass_rust.SyncInfo(on_wait=waits, on_update=[])
                pre.append(nop)
                waits = []
            inst.sync_info = bass_rust.SyncInfo(on_wait=waits, on_update=ups)
            insts[i:i] = pre
            i += len(pre)
            if post:
                insts[i + 1 : i + 1] = post
            n_split += 1
            i += 1
    return n_split


def _build(reps=1, loop_reps=1, mode="full"):
    import concourse.bass as bass
    import concourse.mybir as mybir
    from concourse.tile import TileContext

    nc = bass.Bass(trn_type="TRN2")
    f32 = mybir.dt.float32
    f32r = mybir.dt.float32r
    u8 = mybir.dt.uint8

    emb_t = nc.dram_tensor("emb_t", [H, N], f32, kind="ExternalInput")
    emb_rows_t = nc.dram_tensor("emb_rows_t", [H, ROWS], f32, kind="ExternalInput")
    w1a_t = nc.dram_tensor("w1a_t", [H, H], f32, kind="ExternalInput")
    w1b_t = nc.dram_tensor("w1b_t", [H, H], f32, kind="ExternalInput")
    b1_col = nc.dram_tensor("b1_col", [H, 1], f32, kind="ExternalInput")
    zbuf = nc.dram_tensor("zbuf", [H, 2 * H], f32, kind="ExternalInput")
    b2_col = nc.dram_tensor("b2_col", [H, 1], f32, kind="ExternalInput")
    negb2_col = nc.dram_tensor("negb2_col", [H, 1], f32, kind="ExternalInput")
    # rowcol[k] = global row index of local row k: used to build the one-hot
    # eye matrix on device (iota + is_equal) that injects -BIG into the
    # diagonal score entries via one accumulating matmul
    rowcol = nc.dram_tensor("rowcol", [ROWS, 1], f32, kind="ExternalInput")
    negbig_eye = nc.dram_tensor("negbig_eye", [H, H], f32, kind="ExternalInput")

    bf16 = mybir.dt.bfloat16
    # weights leave the core as bf16 (halves the tail DMA); host upcasts.
    # Adds ~1e-3 rel err on weights vs the 2e-2 gate.
    w_out = nc.dram_tensor("w_out", [ROWS, N], bf16, kind="ExternalOutput")
    m_out = nc.dram_tensor("m_out", [ROWS, N], u8, kind="ExternalOutput")

    with TileContext(nc) as tc:
        with (
            tc.tile_pool(name="const", bufs=1) as cp,
            tc.tile_pool(name="xp", bufs=14) as xp,
            tc.tile_pool(name="pp", bufs=1, space="PSUM") as pp,
        ):
            emba_s = cp.tile([H, 512], f32, tag="emba")
            nc.sync.dma_start(out=emba_s[:], in_=emb_t[:, 0:512])
            embb_s = cp.tile([H, 512], f32, tag="embb")
            nc.sync.dma_start(out=embb_s[:], in_=emb_t[:, 512:1024])
            embr_s = cp.tile([H, ROWS], f32, tag="embr")
            nc.sync.dma_start(out=embr_s[:], in_=emb_rows_t[:])
            w1a_s = cp.tile([H, H], f32, tag="w1a")
            nc.sync.dma_start(out=w1a_s[:], in_=w1a_t[:])
            w1b_s = cp.tile([H, H], f32, tag="w1b")
            nc.sync.dma_start(out=w1b_s[:], in_=w1b_t[:])
            b1_s = cp.tile([H, 1], f32, tag="b1")
            nc.sync.dma_start(out=b1_s[:], in_=b1_col[:])
            z_s = cp.tile([H, 2 * H], f32, tag="z")
            nc.sync.dma_start(out=z_s[:], in_=zbuf[:])
            b2_s = cp.tile([H, 1], f32, tag="b2")
            nc.sync.dma_start(out=b2_s[:], in_=b2_col[:])
            nb2_s = cp.tile([H, 1], f32, tag="nb2")
            nc.sync.dma_start(out=nb2_s[:], in_=negb2_col[:])
            rc_s = cp.tile([ROWS, 1], f32, tag="rc")
            nc.sync.dma_start(out=rc_s[:], in_=rowcol[:])
            nbe_s = cp.tile([H, H], f32, tag="nbe")
            nc.sync.dma_start(out=nbe_s[:], in_=negbig_eye[:])

            # round f32r constants once
            zr_s = cp.tile([H, 2 * H], f32r, tag="zr")
            nc.vector.tensor_copy(zr_s[:], z_s[:])
            nber_s = cp.tile([H, H], f32r, tag="nber")
            nc.vector.tensor_copy(nber_s[:], nbe_s[:])
            # f32r copies of emb / W1 halves: lets BT/CT run as 1-cycle/col
            # f32r matmuls instead of 4-cycle/col f32 (prologue-only cost)
            embar_s = cp.tile([H, 512], f32r, tag="embar")
            nc.vector.tensor_copy(embar_s[:], emba_s[:])
            embbr_s = cp.tile([H, 512], f32r, tag="embbr")
            nc.vector.tensor_copy(embbr_s[:], embb_s[:])
            embrr_s = cp.tile([H, ROWS], f32r, tag="embrr")
            nc.vector.tensor_copy(embrr_s[:], embr_s[:])
            w1ar_s = cp.tile([H, H], f32r, tag="w1ar")
            nc.vector.tensor_copy(w1ar_s[:], w1a_s[:])
            w1br_s = cp.tile([H, H], f32r, tag="w1br")
            nc.vector.tensor_copy(w1br_s[:], w1b_s[:])

            # build the one-hot eye matrix on device: eyr[k, j] = (j == rowcol[k])
            it_s = cp.tile([ROWS, N], f32, tag="it")
            nc.gpsimd.iota(it_s[:], pattern=[[1, N]], base=0,
                           channel_multiplier=0,
                           allow_small_or_imprecise_dtypes=True)
            eyr_s = cp.tile([ROWS, N], f32r, tag="eyr")
            nc.vector.tensor_scalar(
                out=eyr_s[:],
                in0=it_s[:],
                scalar1=rc_s[:],
                scalar2=None,
                op0=mybir.AluOpType.is_equal,
            )

            # warm the PE HAM (clock gate) with dummy f32 matmuls while the
            # large input DMAs land, so prep + early main-loop matmuls run at
            # 2.4 GHz instead of the cold 1.2 GHz
            warm_ps = pp.tile([H, 128], f32, tag="warmp")
            for _w in range(12):
                nc.tensor.matmul(
                    warm_ps[:], w1a_s[:], w1a_s[:], start=True, stop=True
                )

            # force the sigmoid ACT table set to load during prep, so the
            # epilogue sigmoid doesn't pay a ~2.7us mid-kernel table swap
            # (relu/identity are filler entries in every set); reading
            # warm_ps also keeps the warm matmuls alive through DCE
            warm_s = cp.tile([H, 1], f32, tag="warm")
            nc.scalar.activation(
                warm_s[:], warm_ps[:, 0:1], mybir.ActivationFunctionType.Sigmoid
            )

            if mode in ("full", "v4", "v4s", "v5"):
                # software-pipelined: BT/CT/eye-init are rep-invariant, so
                # each body computes them for the NEXT rep during the DMA
                # tail; gens read the copies produced by the previous rep
                ct_ps = pp.tile([H, ROWS], f32, tag="ctp")
                ct_s = cp.tile([H, ROWS], f32, tag="ct")
                bt_ps = pp.tile([H, N], f32, tag="btp")
                bt_s = cp.tile([H, N], f32, tag="bt")
                sc_ps = pp.tile([ROWS, N], f32, tag="scores")
                sig_s = cp.tile([ROWS, N], mybir.dt.bfloat16, tag="sig")
                m_s = cp.tile([ROWS, N], u8, tag="m")

                def prep():
                    nc.tensor.matmul(ct_ps[:], w1ar_s[:], embrr_s[:],
                                     start=True, stop=True)
                    nc.scalar.activation(
                        ct_s[:], ct_ps[:],
                        mybir.ActivationFunctionType.Identity, bias=b1_s[:])
                    nc.tensor.matmul(bt_ps[:, 0:512], w1br_s[:], embar_s[:],
                                     start=True, stop=True)
                    nc.tensor.matmul(bt_ps[:, 512:1024], w1br_s[:], embbr_s[:],
                                     start=True, stop=True)
                    nc.vector.tensor_copy(bt_s[:, 0:512], bt_ps[:, 0:512])
                    nc.vector.tensor_copy(bt_s[:, 512:1024], bt_ps[:, 512:1024])
                    for h0 in (0, 512):
                        nc.tensor.matmul(
                            sc_ps[:, h0 : h0 + 512], nber_s[:],
                            eyr_s[:, h0 : h0 + 512],
                            start=True, stop=False, skip_group_check=True)

                def body():
                    for i in range(ROWS):
                        x = xp.tile([H, N], f32r, tag="x")
                        if i % 3 == 1:
                            nc.scalar.activation(
                                x[:], bt_s[:],
                                mybir.ActivationFunctionType.Relu,
                                bias=ct_s[:, i : i + 1])
                        else:
                            nc.vector.tensor_scalar(
                                out=x[:], in0=bt_s[:],
                                scalar1=ct_s[:, i : i + 1], scalar2=0.0,
                                op0=mybir.AluOpType.add,
                                op1=mybir.AluOpType.max)
                        lhsT = zr_s[:, H - i : 2 * H - i]
                        nc.tensor.matmul(
                            sc_ps[:, 0:512], lhsT, x[:, 0:512],
                            start=False, stop=(i == ROWS - 1),
                            skip_group_check=True)
                        nc.tensor.matmul(
                            sc_ps[:, 512:1024], lhsT, x[:, 512:1024],
                            start=False, stop=(i == ROWS - 1),
                            skip_group_check=True)
                    if mode != "v5":
                        for h0 in (0, 512):
                            nc.scalar.activation(
                                sig_s[:, h0 : h0 + 512], sc_ps[:, h0 : h0 + 512],
                                mybir.ActivationFunctionType.Sigmoid, bias=b2_s[:])
                            nc.sync.dma_start(out=w_out[:, h0 : h0 + 512],
                                              in_=sig_s[:, h0 : h0 + 512])
                        nc.vector.tensor_scalar(
                            out=m_s[:], in0=sc_ps[:], scalar1=nb2_s[:],
                            scalar2=None, op0=mybir.AluOpType.is_gt)
                        nc.sync.dma_start(out=m_out[:], in_=m_s[:])
                        prep()
                        return
                    # v5: engine-queue-aware ordering of tail + next-rep prep:
                    # DVE [gens, btcopies, is_gt], ACT [gens, sigmoids, ct],
                    # PE [MMs, BT, CT, eye] so next-rep inputs land earliest
                    for h0 in (0, 512):
                        nc.scalar.activation(
                            sig_s[:, h0 : h0 + 512], sc_ps[:, h0 : h0 + 512],
                            mybir.ActivationFunctionType.Sigmoid, bias=b2_s[:])
                        nc.sync.dma_start(out=w_out[:, h0 : h0 + 512],
                                          in_=sig_s[:, h0 : h0 + 512])
                    nc.tensor.matmul(bt_ps[:, 0:512], w1br_s[:], embar_s[:],
                                     start=True, stop=True)
                    nc.tensor.matmul(bt_ps[:, 512:1024], w1br_s[:], embbr_s[:],
                                     start=True, stop=True)
                    nc.vector.tensor_copy(bt_s[:, 0:512], bt_ps[:, 0:512])
                    nc.vector.tensor_copy(bt_s[:, 512:1024], bt_ps[:, 512:1024])
                    nc.tensor.matmul(ct_ps[:], w1ar_s[:], embrr_s[:],
                                     start=True, stop=True)
                    nc.vector.tensor_scalar(
                        out=m_s[:], in0=sc_ps[:], scalar1=nb2_s[:],
                        scalar2=None, op0=mybir.AluOpType.is_gt)
                    nc.sync.dma_start(out=m_out[:], in_=m_s[:])
                    nc.scalar.activation(
                        ct_s[:], ct_ps[:],
                        mybir.ActivationFunctionType.Identity, bias=b1_s[:])
                    for h0 in (0, 512):
                        nc.tensor.matmul(
                            sc_ps[:, h0 : h0 + 512], nber_s[:],
                            eyr_s[:, h0 : h0 + 512],
                            start=True, stop=False, skip_group_check=True)

                prep()
            elif mode == "v3":
                def body():
                    _body_v3(nc, tc, cp, xp, pp, mybir, f32, f32r, u8,
                             embar_s, embbr_s, embrr_s, w1ar_s, w1br_s, b1_s,
                             zr_s, b2_s, nb2_s, eyr_s, nber_s, w_out, m_out)
            elif mode in ("v2psum", "v2sb"):
                def body():
                    _body_v2(nc, tc, cp, xp, pp, mybir, f32, f32r, u8,
                             embar_s, embbr_s, embrr_s, w1ar_s, w1br_s, b1_s,
                             zr_s, b2_s, nb2_s, eyr_s, nber_s, w_out, m_out,
                             act_src_psum=(mode == "v2psum"))
            else:
                def body():
                    _body_once(nc, tc, cp, xp, pp, mybir, f32, f32r, u8,
                               (emba_s, embb_s), embr_s, w1a_s, w1b_s, b1_s,
                               zr_s, b2_s, nb2_s, eyr_s, nber_s, w_out, m_out,
                               mode)

            if loop_reps > 1:
                # staggered_reset drops the per-iteration all-engine barrier
                # (rolling per-stage sem resets instead), letting engines flow
                # into the next rep while others drain the tail
                with tc.For_i(0, loop_reps, 1,
                              staggered_reset=(mode == "v4s")):
                    body()
            else:
                for _rep in range(reps):
                    body()

    _split_multiwaits(nc)
    return nc


def _body_v2(nc, tc, cp, xp, pp, mybir, f32, f32r, u8,
             embar_s, embbr_s, embrr_s, w1ar_s, w1br_s, b1_s, zr_s, b2_s,
             nb2_s, eyr_s, nber_s, w_out, m_out, act_src_psum=True):
    """PE-rate-bound body: f32r BT/CT, ACT gens read bt straight from PSUM,
    DVE gens read an SBUF copy; pipelined epilogue halves."""
    # BT = W1b @ embT (f32r matmuls: 1 cyc/col instead of 4)
    bt_ps = pp.tile([H, N], f32, tag="btp")
    nc.tensor.matmul(bt_ps[:, 0:512], w1br_s[:], embar_s[:], start=True, stop=True)
    nc.tensor.matmul(bt_ps[:, 512:1024], w1br_s[:], embbr_s[:], start=True, stop=True)
    # CT = W1a @ embT_rows + b1
    ct_ps = pp.tile([H, ROWS], f32, tag="ctp")
    nc.tensor.matmul(ct_ps[:], w1ar_s[:], embrr_s[:], start=True, stop=True)
    ct_s = cp.tile([H, ROWS], f32, tag="ct")
    nc.scalar.activation(
        ct_s[:], ct_ps[:], mybir.ActivationFunctionType.Identity, bias=b1_s[:]
    )
    # SBUF copy of bt for the DVE generators (DVE from PSUM would drop to 1x)
    bt_s = cp.tile([H, N], f32, tag="bt")
    nc.vector.tensor_copy(bt_s[:], bt_ps[:])

    # scores PSUM, diagonal pre-initialized to -BIG
    sc_ps = pp.tile([ROWS, N], f32, tag="scores")
    for h0 in (0, 512):
        nc.tensor.matmul(
            sc_ps[:, h0 : h0 + 512], nber_s[:], eyr_s[:, h0 : h0 + 512],
            start=True, stop=False,
        )

    # main loop: ACT tiles early (ACT is ready before the bt SBUF copy lands),
    # then interleave so both engines stay fed; ACT reads bt from PSUM
    # ((N+172)/1.2 vs (N+352)/1.2 from SBUF)
    act_set = {0, 1}
    act_set.update(i for i in range(2, ROWS) if i % 3 == 2)
    for i in range(ROWS):
        x = xp.tile([H, N], f32r, tag="x")
        if i in act_set:
            nc.scalar.activation(
                x[:], bt_ps[:] if act_src_psum else bt_s[:],
                mybir.ActivationFunctionType.Relu,
                bias=ct_s[:, i : i + 1],
            )
        else:
            nc.vector.tensor_scalar(
                out=x[:], in0=bt_s[:],
                scalar1=ct_s[:, i : i + 1], scalar2=0.0,
                op0=mybir.AluOpType.add, op1=mybir.AluOpType.max,
            )
        lhsT = zr_s[:, H - i : 2 * H - i]
        nc.tensor.matmul(
            sc_ps[:, 0:512], lhsT, x[:, 0:512],
            start=False, stop=(i == ROWS - 1),
        )
        nc.tensor.matmul(
            sc_ps[:, 512:1024], lhsT, x[:, 512:1024],
            start=False, stop=(i == ROWS - 1),
        )

    # epilogue: halves so the first w_out DMA overlaps the second sigmoid
    sig_s = cp.tile([ROWS, N], mybir.dt.bfloat16, tag="sig")
    for h0 in (0, 512):
        nc.scalar.activation(
            sig_s[:, h0 : h0 + 512], sc_ps[:, h0 : h0 + 512],
            mybir.ActivationFunctionType.Sigmoid, bias=b2_s[:],
        )
        nc.sync.dma_start(out=w_out[:, h0 : h0 + 512], in_=sig_s[:, h0 : h0 + 512])
    m_s = cp.tile([ROWS, N], u8, tag="m")
    nc.vector.tensor_scalar(
        out=m_s[:], in0=sc_ps[:], scalar1=nb2_s[:], scalar2=None,
        op0=mybir.AluOpType.is_gt,
    )
    nc.sync.dma_start(out=m_out[:], in_=m_s[:])


def _body_v3(nc, tc, cp, xp, pp, mybir, f32, f32r, u8,
             embar_s, embbr_s, embrr_s, w1ar_s, w1br_s, b1_s, zr_s, b2_s,
             nb2_s, eyr_s, nber_s, w_out, m_out):
    """fullold dataflow (SBUF-src gens, split bt copies on DVE+ACT) with:
    CT-first head, f32r BT/CT matmuls, pipelined sigmoid/DMA tail."""
    # CT first so ct_s is ready before the first ACT generation
    ct_ps = pp.tile([H, ROWS], f32, tag="ctp")
    nc.tensor.matmul(ct_ps[:], w1ar_s[:], embrr_s[:], start=True, stop=True)
    ct_s = cp.tile([H, ROWS], f32, tag="ct")
    nc.scalar.activation(
        ct_s[:], ct_ps[:], mybir.ActivationFunctionType.Identity, bias=b1_s[:]
    )
    bt_ps = pp.tile([H, N], f32, tag="btp")
    nc.tensor.matmul(bt_ps[:, 0:512], w1br_s[:], embar_s[:], start=True, stop=True)
    nc.tensor.matmul(bt_ps[:, 512:1024], w1br_s[:], embbr_s[:], start=True, stop=True)
    bt_s = cp.tile([H, N], f32, tag="bt")
    nc.vector.tensor_copy(bt_s[:, 0:512], bt_ps[:, 0:512])
    nc.scalar.copy(bt_s[:, 512:1024], bt_ps[:, 512:1024])

    sc_ps = pp.tile([ROWS, N], f32, tag="scores")
    for h0 in (0, 512):
        nc.tensor.matmul(
            sc_ps[:, h0 : h0 + 512], nber_s[:], eyr_s[:, h0 : h0 + 512],
            start=True, stop=False,
        )

    for i in range(ROWS):
        x = xp.tile([H, N], f32r, tag="x")
        if i % 3 == 1:
            nc.scalar.activation(
                x[:], bt_s[:], mybir.ActivationFunctionType.Relu,
                bias=ct_s[:, i : i + 1],
            )
        else:
            nc.vector.tensor_scalar(
                out=x[:], in0=bt_s[:],
                scalar1=ct_s[:, i : i + 1], scalar2=0.0,
                op0=mybir.AluOpType.add, op1=mybir.AluOpType.max,
            )
        lhsT = zr_s[:, H - i : 2 * H - i]
        nc.tensor.matmul(
            sc_ps[:, 0:512], lhsT, x[:, 0:512],
            start=False, stop=(i == ROWS - 1),
        )
        nc.tensor.matmul(
            sc_ps[:, 512:1024], lhsT, x[:, 512:1024],
            start=False, stop=(i == ROWS - 1),
        )

    sig_s = cp.tile([ROWS, N], mybir.dt.bfloat16, tag="sig")
    for h0 in (0, 512):
        nc.scalar.activation(
            sig_s[:, h0 : h0 + 512], sc_ps[:, h0 : h0 + 512],
            mybir.ActivationFunctionType.Sigmoid, bias=b2_s[:],
        )
        nc.sync.dma_start(out=w_out[:, h0 : h0 + 512], in_=sig_s[:, h0 : h0 + 512])
    m_s = cp.tile([ROWS, N], u8, tag="m")
    nc.vector.tensor_scalar(
        out=m_s[:], in0=sc_ps[:], scalar1=nb2_s[:], scalar2=None,
        op0=mybir.AluOpType.is_gt,
    )
    nc.sync.dma_start(out=m_out[:], in_=m_s[:])


def _body_once(nc, tc, cp, xp, pp, mybir, f32, f32r, u8,
               embt_halves, embr_s, w1a_s, w1b_s, b1_s, zr_s, b2_s, nb2_s,
               eyr_s, nber_s, w_out, m_out, mode="full"):
    emba_s, embb_s = embt_halves
    if mode == "empty":
        return
    if True:
        if True:
            # BT = W1b @ embT  (f32, exact): psum half per matmul; each half
            # depends only on its own emb DMA, and the PSUM->SBUF copies run
            # on different engines so they overlap
            bt_ps = pp.tile([H, N], f32, tag="btp")
            nc.tensor.matmul(
                bt_ps[:, 0:512], w1b_s[:], emba_s[:], start=True, stop=True
            )
            nc.tensor.matmul(
                bt_ps[:, 512:1024], w1b_s[:], embb_s[:], start=True, stop=True
            )
            bt_s = cp.tile([H, N], f32, tag="bt")
            nc.vector.tensor_copy(bt_s[:, 0:512], bt_ps[:, 0:512])
            nc.scalar.copy(bt_s[:, 512:1024], bt_ps[:, 512:1024])

            # CT = W1a @ embT_rows + b1  (f32, exact)
            ct_ps = pp.tile([H, ROWS], f32, tag="ctp")
            nc.tensor.matmul(ct_ps[:], w1a_s[:], embr_s[:], start=True, stop=True)
            ct_s = cp.tile([H, ROWS], f32, tag="ct")
            nc.scalar.activation(
                ct_s[:], ct_ps[:], mybir.ActivationFunctionType.Identity, bias=b1_s[:]
            )

            # main loop: accumulate scores into PSUM [128 rows, 1024 cols]
            sc_ps = pp.tile([ROWS, N], f32, tag="scores")
            # initialize each scores bank with -BIG at the diagonal entries
            # (zeros elsewhere): out[k, j] = -BIG*eye[k, j]; keeps the
            # epilogue off the critical tail
            for h0 in (0, 512):
                nc.tensor.matmul(
                    sc_ps[:, h0 : h0 + 512],
                    nber_s[:],
                    eyr_s[:, h0 : h0 + 512],
                    start=True,
                    stop=False,
                )
            xfix = None
            if mode in ("nogen", "nogen_fixw", "nogen_w32", "nogen_1bank",
                        "nogen_fixw32", "nogen_b8", "nogen_256"):
                xfix = cp.tile([H, N], f32r, tag="xfix")
                nc.vector.tensor_copy(xfix[:, 0:256], zr_s[:])
            if mode == "nogen_256":
                # same total moving cols as nogen, but 512 MMs x 256 cols:
                # separates per-MM fixed overhead from cycle-rate
                lhsT = zr_s[:, 0:H]
                for i in range(2 * ROWS):
                    for c0 in (0, 256):
                        nc.tensor.matmul(
                            sc_ps[:, c0 : c0 + 256], lhsT, xfix[:, c0 : c0 + 256],
                            start=False, stop=(i == 2 * ROWS - 1 and c0 == 256),
                        )
                _epilogue(nc, cp, mybir, f32, u8, sc_ps, b2_s, nb2_s, w_out, m_out)
                return
            if mode == "nogen_bf16":
                # 256 MMs x 512 cols with bf16 moving + bf16 stationary:
                # tests whether the ~250ns/MM is f32r-specific or clock/overhead
                bf16 = mybir.dt.bfloat16
                xbf = cp.tile([H, N], bf16, tag="xbf")
                nc.vector.tensor_copy(xbf[:, 0:256], zr_s[:])
                zbf = cp.tile([H, H], bf16, tag="zbf")
                nc.vector.tensor_copy(zbf[:], zr_s[:, 0:H])
                for i in range(2 * ROWS):
                    nc.tensor.matmul(
                        sc_ps[:, 0:512], zbf[:], xbf[:, 0:512],
                        start=False, stop=(i == 2 * ROWS - 1),
                    )
                _epilogue(nc, cp, mybir, f32, u8, sc_ps, b2_s, nb2_s, w_out, m_out)
                return
            if mode == "nogen_1bank":
                # fixed 128-wide stationary, all MMs -> one PSUM bank
                lhsT = zr_s[:, 0:H]
                for i in range(2 * ROWS):
                    nc.tensor.matmul(
                        sc_ps[:, 0:512], lhsT, xfix[:, 0:512],
                        start=False, stop=(i == 2 * ROWS - 1),
                    )
                _epilogue(nc, cp, mybir, f32, u8, sc_ps, b2_s, nb2_s, w_out, m_out)
                return
            if mode == "nogen_fixw32":
                # fixed 32-wide stationary, all MMs -> one PSUM region
                lhsT = zr_s[:, 128:160]
                for i in range(2 * ROWS):
                    nc.tensor.matmul(
                        sc_ps[0:32, 0:512], lhsT, xfix[:, 0:512],
                        start=False, stop=(i == 2 * ROWS - 1),
                    )
                _epilogue(nc, cp, mybir, f32, u8, sc_ps, b2_s, nb2_s, w_out, m_out)
                return
            if mode == "nogen_b8":
                # sliding stationaries, banks switched every 8 rows
                for i0 in range(0, ROWS, 8):
                    for h0 in (0, 512):
                        for i in range(i0, i0 + 8):
                            lhsT = zr_s[:, H - i : 2 * H - i]
                            nc.tensor.matmul(
                                sc_ps[:, h0 : h0 + 512], lhsT, xfix[:, h0 : h0 + 512],
                                start=False,
                                stop=(i == ROWS - 1),
                            )
                _epilogue(nc, cp, mybir, f32, u8, sc_ps, b2_s, nb2_s, w_out, m_out)
                return
            if mode == "nogen_fixw":
                # PE-only, FIXED stationary: isolates LDWEIGHTS cost vs nogen
                lhsT = zr_s[:, 0:H]
                for i in range(ROWS):
                    nc.tensor.matmul(
                        sc_ps[:, 0:512], lhsT, xfix[:, 0:512],
                        start=False, stop=(i == ROWS - 1),
                    )
                    nc.tensor.matmul(
                        sc_ps[:, 512:1024], lhsT, xfix[:, 512:1024],
                        start=False, stop=(i == ROWS - 1),
                    )
                _epilogue(nc, cp, mybir, f32, u8, sc_ps, b2_s, nb2_s, w_out, m_out)
                return
            if mode == "nogen_w32":
                # PE-only, 32-wide sliding stationaries + tile_position groups
                for g in range(4):
                    for k in range(32):
                        lhsT = zr_s[:, H - k : H + 32 - k]
                        for h0 in (0, 512):
                            nc.tensor.matmul(
                                sc_ps[32 * g : 32 * g + 32, h0 : h0 + 512],
                                lhsT,
                                xfix[:, h0 : h0 + 512],
                                start=False,
                                stop=(k == 31),
                                tile_position=(0, 32 * g),
                            )
                _epilogue(nc, cp, mybir, f32, u8, sc_ps, b2_s, nb2_s, w_out, m_out)
                return
            if mode == "full2":
                # col-group tiled reduction: 32-wide stationaries, 4 strips
                for k in range(32):
                    for g in range(4):
                        i = 32 * g + k
                        x = xp.tile([H, N], f32r, tag="x")
                        if (i * 5) % 13 < 5:
                            nc.scalar.activation(
                                x[:],
                                bt_s[:],
                                mybir.ActivationFunctionType.Relu,
                                bias=ct_s[:, i : i + 1],
                            )
                        else:
                            nc.vector.tensor_scalar(
                                out=x[:],
                                in0=bt_s[:],
                                scalar1=ct_s[:, i : i + 1],
                                scalar2=0.0,
                                op0=mybir.AluOpType.add,
                                op1=mybir.AluOpType.max,
                            )
                        lhsT = zr_s[:, H - k : H + 32 - k]
                        for h0 in (0, 512):
                            nc.tensor.matmul(
                                sc_ps[32 * g : 32 * g + 32, h0 : h0 + 512],
                                lhsT,
                                x[:, h0 : h0 + 512],
                                start=(k == 0),
                                stop=(k == 31),
                                tile_position=(0, 32 * g),
                            )
                _epilogue(nc, cp, mybir, f32, u8, sc_ps, b2_s, nb2_s, w_out, m_out)
                return

            for i in range(ROWS):
                if mode != "nogen":
                    x = xp.tile([H, N], f32r, tag="x")
                    if mode == "actgen" or (mode != "dvegen" and i % 3 == 1):
                        # ACT path: relu(in + bias), ~1147ns
                        nc.scalar.activation(
                            x[:],
                            bt_s[:],
                            mybir.ActivationFunctionType.Relu,
                            bias=ct_s[:, i : i + 1],
                        )
                    else:
                        # DVE path: (in + c_i) then max(.,0), ~720ns
                        nc.vector.tensor_scalar(
                            out=x[:],
                            in0=bt_s[:],
                            scalar1=ct_s[:, i : i + 1],
                            scalar2=0.0,
                            op0=mybir.AluOpType.add,
                            op1=mybir.AluOpType.max,
                        )
                else:
                    x = xfix
                if mode == "nomm":
                    continue
                lhsT = zr_s[:, H - i : 2 * H - i]
                nc.tensor.matmul(
                    sc_ps[:, 0:512],
                    lhsT,
                    x[:, 0:512],
                    start=False,
                    stop=(i == ROWS - 1),
                )
                nc.tensor.matmul(
                    sc_ps[:, 512:1024],
                    lhsT,
                    x[:, 512:1024],
                    start=False,
                    stop=(i == ROWS - 1),
                )
            if mode == "nomm":
                return

            _epilogue(nc, cp, mybir, f32, u8, sc_ps, b2_s, nb2_s, w_out, m_out)


def _epilogue(nc, cp, mybir, f32, u8, sc_ps, b2_s, nb2_s, w_out, m_out):
    # diagonal score entries hold -BIG: sigmoid -> 0 weight, is_gt -> 0 mask
    sig_s = cp.tile([ROWS, N], mybir.dt.bfloat16, tag="sig")
    nc.scalar.activation(
        sig_s[:], sc_ps[:], mybir.ActivationFunctionType.Sigmoid, bias=b2_s[:]
    )
    nc.sync.dma_start(out=w_out[:], in_=sig_s[:])

    m_s = cp.tile([ROWS, N], u8, tag="m")
    nc.vector.tensor_scalar(
        out=m_s[:],
        in0=sc_ps[:],
        scalar1=nb2_s[:],
        scalar2=None,
        op0=mybir.AluOpType.is_gt,
    )
    nc.sync.dma_start(out=m_out[:], in_=m_s[:])


def _build_in_maps(inputs):
    node_emb = np.asarray(inputs["node_emb"], dtype=np.float32)
    W1 = np.asarray(inputs["W1"], dtype=np.float32)
    b1 = np.asarray(inputs["b1"], dtype=np.float32)
    W2 = np.asarray(inputs["W2"], dtype=np.float32)
    b2 = np.asarray(inputs["b2"], dtype=np.float32)

    emb_t = np.ascontiguousarray(node_emb.T)  # [H, N]
    w1a_t = np.ascontiguousarray(W1[:, :H].T)  # [e, h]
    w1b_t = np.ascontiguousarray(W1[:, H:].T)
    b1_col = np.ascontiguousarray(b1.reshape(H, 1))
    zbuf = np.zeros((H, 2 * H), dtype=np.float32)
    zbuf[:, H] = W2[0]
    b2v = np.float32(b2.reshape(-1)[0])
    b2_col = np.full((H, 1), b2v, dtype=np.float32)
    negb2_col = -b2_col

    negbig_eye = np.zeros((H, H), dtype=np.float32)
    np.fill_diagonal(negbig_eye, np.float32(-1e30))

    in_maps = []
    for c in range(NCORES):
        r0 = c * ROWS
        in_maps.append(
            {
                "emb_t": emb_t,
                "emb_rows_t": np.ascontiguousarray(emb_t[:, r0 : r0 + ROWS]),
                "w1a_t": w1a_t,
                "w1b_t": w1b_t,
                "b1_col": b1_col,
                "zbuf": zbuf,
                "b2_col": b2_col,
                "negb2_col": negb2_col,
                "rowcol": (r0 + np.arange(ROWS, dtype=np.float32)).reshape(ROWS, 1),
                "negbig_eye": negbig_eye,
            }
        )
    return in_maps


def _make_runner(nc):
    """Build a reusable jitted runner (mirrors bass2jax.run_bass_via_pjrt,
    but cached so repeated kernel() calls skip re-tracing/compiling)."""
    import jax
    import concourse.mybir as mybir
    from jax.sharding import Mesh, PartitionSpec

    try:
        from jax.experimental.shard_map import shard_map
    except ImportError:
        from jax.shard_map import shard_map

    from concourse.bass2jax import (
        _bass_exec_p,
        install_neuronx_cc_hook,
        partition_id_tensor,
    )

    install_neuronx_cc_hook()
    partition_name = nc.partition_id_tensor.name if nc.partition_id_tensor else None

    in_names, out_names, out_avals, zero_outs = [], [], [], []
    for alloc in nc.m.functions[0].allocations:
        if not isinstance(alloc, mybir.MemoryLocationSet):
            continue
        name = alloc.memorylocations[0].name
        if alloc.kind == "ExternalInput":
            if name != partition_name:
                in_names.append(name)
        elif alloc.kind == "ExternalOutput":
            out_names.append(name)
            shape = tuple(alloc.tensor_shape)
            dtype = mybir.dt.np(alloc.dtype)
            out_avals.append(jax.core.ShapedArray(shape, dtype))
            zero_outs.append(np.zeros(shape, dtype))
    n_params = len(in_names)
    all_in_names = list(in_names) + list(out_names)
    if partition_name is not None:
        all_in_names.append(partition_name)

    def _body(*args):
        operands = list(args)
        if partition_name is not None:
            operands.append(partition_id_tensor())
        return tuple(
            _bass_exec_p.bind(
                *operands,
                out_avals=tuple(out_avals),
                in_names=tuple(all_in_names),
                out_names=tuple(out_names),
                lowering_input_output_aliases=(),
                sim_require_finite=True,
                sim_require_nnan=True,
                nc=nc,
            )
        )

    devices = jax.devices()[:NCORES]
    mesh = Mesh(np.asarray(devices), ("core",))
    n_outs = len(out_avals)
    # only these inputs differ per core; the rest are replicated and ship
    # to the devices once instead of 8 concatenated copies
    per_core_names = {"emb_rows_t", "rowcol"}
    in_specs = tuple(
        PartitionSpec("core") if n in per_core_names else PartitionSpec(None)
        for n in in_names
    ) + (PartitionSpec("core"),) * n_outs
    out_specs = (PartitionSpec("core"),) * n_outs
    fn = jax.jit(
        shard_map(_body, mesh=mesh, in_specs=in_specs, out_specs=out_specs,
                  check_rep=False),
        keep_unused=True,
    )
    concat_zeros = [
        np.zeros((NCORES * z.shape[0], *z.shape[1:]), z.dtype) for z in zero_outs
    ]
    return fn, in_names, out_names, out_avals, concat_zeros, per_core_names


def _run_cached(in_maps):
    import jax

    if "runner" not in _cache:
        _cache["runner"] = _make_runner(_cache["nc"])
    fn, in_names, out_names, out_avals, concat_zeros, per_core_names = _cache["runner"]
    concat_in = [
        np.concatenate([np.asarray(m[name]) for m in in_maps], axis=0)
        if name in per_core_names
        else np.asarray(in_maps[0][name])
        for name in in_names
    ]
    out_arrs = fn(*concat_in, *concat_zeros)
    jax.block_until_ready(out_arrs)
    res = {}
    for i, name in enumerate(out_names):
        res[name] = np.asarray(out_arrs[i]).reshape(
            NCORES, *out_avals[i].shape
        )
    return res


def kernel(node_emb, W1, b1, W2, b2, temperature=None, **_ignored):
    import time

    if "nc" not in _cache:
        _cache["nc"] = _build()

    in_maps = _build_in_maps(
        {"node_emb": node_emb, "W1": W1, "b1": b1, "W2": W2, "b2": b2}
    )
    # the device occasionally reports NRT_EXEC_UNIT_UNRECOVERABLE if a prior
    # process wedged it; it self-recovers after ~30s, so retry those (and only
    # those) with backoff
    for attempt in range(3):
        try:
            res = _run_cached(in_maps)
            break
        except Exception as e:  # noqa: BLE001
            msg = str(e)
            transient = (
                "UNRECOVERABLE" in msg
                or "unrecoverable" in msg
                or "UNAVAILABLE" in msg
            )
            if attempt == 2 or not transient:
                raise
            time.sleep(30 * (attempt + 1))
    weights = np.concatenate(
        [np.asarray(res["w_out"][c]).astype(np.float32) for c in range(NCORES)],
        axis=0,
    )
    mask = np.concatenate([res["m_out"][c] for c in range(NCORES)], axis=0).astype(bool)
    return weights, mask



# revision 46
# speedup vs baseline: 1.0598x; 1.0598x over previous
"""Distributed TRN2 Bass kernel for AdaptiveGraphTopology pairwise edge MLP.

reference:
    a = emb @ W1a.T ; b = emb @ W1b.T           (W1a, W1b = W1[:, :H], W1[:, H:])
    hidden = relu(a[:,None,:] + b[None,:,:] + b1)      # [N,N,H]
    scores = hidden . W2[0] + b2                       # [N,N]
    weights = sigmoid(scores), zeroed diag
    mask    = (weights > 0.5) & ~eye

Sharding: rows i split across 8 cores (128 rows each); everything else
replicated. No collectives: each core DMAs out its row block, host
concatenates.

Per-core compute (mode "full" = v4, software-pipelined):
    BT[h, j] = b_j[h]        (all j)    -- f32r matmul on device
    CT[h, i] = a_i[h]+b1[h]  (local i)  -- f32r matmul + bias on device
    loop over local i:
      X_i[h, j] = relu(BT[h, j] + CT[h, i])   (DVE/ACT, fp32r out)
      scores[i, :] += w2 . X_i  via fp32r matmul whose stationary is a
      sliding window over Z[128, 256] (w2 at column 128, zeros elsewhere):
      window [128-i : 256-i] places w2 in PE column i, so row i's scores
      land in PSUM partition i and the 128 iterations accumulate a full
      [128, 1024] score block (zero columns contribute exact zeros).
    The diagonal is pushed to -1e30 by pre-initializing the score PSUM
    with accumulating matmuls (stationary -1e30*I, moving a per-core
    one-hot eye_rows matrix), so the epilogue is just:
    weights = sigmoid(scores+b2) (diag -> 0.0), mask = scores > -b2.

Measured facts (this device) driving the structure:
  - The PE streams moving data at ~2.0 cols/ns regardless of dtype
    (f32r/bf16), stationary width, PSUM bank pattern or MM size: the
    256 x 512-col score matmuls are a hard ~66 us floor; everything
    else must hide behind it.
  - tc.For_i puts an all-engine barrier at each iteration end, so
    cross-rep pipelining is impossible. BT/CT/eye-init are
    rep-invariant, so each body computes them for the NEXT rep during
    its epilogue/DMA tail (PE idle there), and the prologue seeds the
    first rep: the body then starts generating X_0 immediately.
"""
import numpy as np

N = 1024
H = 128
NCORES = 8
ROWS = N // NCORES  # 128 rows per core

# v7 tuning: rows whose scores are computed off-PE, and which gen rows go
# to ACT (pattern periods); see _build mode "v7"
K_OFF = 8
ACT_GEN = {1, 3, 5, 8, 10}  # i % 12 in this set -> ACT gen, else DVE

# v8 tuning: XD rows reduced on DVE, XA rows reduced on ACT (sign-split);
# _V8_PZ = (#positive, #zero) sign counts after the host h-permutation,
# set by _build_in_maps before _build runs
V8_XD = 6
V8_XA = 8
_V8_PZ = [64, 0]

_cache = {}


def _split_multiwaits(nc, limit=1):
    """This walrus build accepts only ONE semaphore wait/update per
    instruction; Tile emits several. Split extras onto adjacent NoOps."""
    import bass_rust

    f = nc.m.functions[0]
    engines = nc.engines

    def make_nop(engine_type):
        eng = engines[engine_type]
        inst = eng.nop(nofuse=True).ins
        for b in f.blocks:
            lst = b.instructions
            for k in range(len(lst) - 1, -1, -1):
                if lst[k] is inst:
                    lst.pop(k)
                    return inst
        return inst

    n_split = 0
    for b in f.blocks:
        insts = b.instructions
        i = 0
        while i < len(insts):
            inst = insts[i]
            si = inst.sync_info
            if si is None:
                i += 1
                continue
            waits = list(si.on_wait)
            ups = list(si.on_update)

            def _is_add_imm(u):
                # sem-add-imm consumes the instruction's immediate field; a
                # wait's compare-immediate then conflicts (walrus
                # no_semaphore_value_conflict). sem-inc (+1) needs no imm.
                return (getattr(u, "update_mode", None) == "sem-add-imm"
                        and getattr(u, "update_value", 1) != 1)

            same_sem = (
                len(waits) >= 1 and len(ups) >= 1
                and (any(getattr(w, "id", None) == getattr(u, "id", None)
                         for w in waits for u in ups)
                     or any(_is_add_imm(u) for u in ups))
            )
            if len(waits) <= limit and len(ups) <= 1 and not same_sem:
                i += 1
                continue
            pre = []
            post = []
            if len(waits) > limit:
                extra, waits = waits[: len(waits) - limit], waits[len(waits) - limit :]
                for w in extra:
                    nop = make_nop(inst.engine)
                    nop.sync_info = bass_rust.SyncInfo(on_wait=[w], on_update=[])
                    pre.append(nop)
            if len(ups) > 1:
                ups, extra_u = ups[:1], ups[1:]
                for u in extra_u:
                    nop = make_nop(inst.engine)
                    nop.sync_info = bass_rust.SyncInfo(on_wait=[], on_update=[u])
                    post.append(nop)
            if (waits and ups
                    and (getattr(waits[0], "id", None) == getattr(ups[0], "id", None)
                         or any(_is_add_imm(u) for u in ups))):
                # wait+update on one semaphore — or a wait-imm next to a
                # sem-add-imm update — trips walrus's
                # no_semaphore_value_conflict: hoist the wait onto a
                # preceding NoOp (engine queues are in-order)
                nop = make_nop(inst.engine)
                nop.sync_info = bass_rust.SyncInfo(on_wait=waits, on_update=[])
                pre.append(nop)
                waits = []
            inst.sync_info = bass_rust.SyncInfo(on_wait=waits, on_update=ups)
            insts[i:i] = pre
            i += len(pre)
            if post:
                insts[i + 1 : i + 1] = post
            n_split += 1
            i += 1
    return n_split


def _build(reps=1, loop_reps=1, mode="v6h"):
    import concourse.bass as bass
    import concourse.mybir as mybir
    from concourse.tile import TileContext

    nc = bass.Bass(trn_type="TRN2")
    f32 = mybir.dt.float32
    f32r = mybir.dt.float32r
    u8 = mybir.dt.uint8

    emb_t = nc.dram_tensor("emb_t", [H, N], f32, kind="ExternalInput")
    emb_rows_t = nc.dram_tensor("emb_rows_t", [H, ROWS], f32, kind="ExternalInput")
    w1a_t = nc.dram_tensor("w1a_t", [H, H], f32, kind="ExternalInput")
    w1b_t = nc.dram_tensor("w1b_t", [H, H], f32, kind="ExternalInput")
    b1_col = nc.dram_tensor("b1_col", [H, 1], f32, kind="ExternalInput")
    zbuf = nc.dram_tensor("zbuf", [H, 2 * H], f32, kind="ExternalInput")
    b2_col = nc.dram_tensor("b2_col", [H, 1], f32, kind="ExternalInput")
    negb2_col = nc.dram_tensor("negb2_col", [H, 1], f32, kind="ExternalInput")
    # rowcol[k] = global row index of local row k: used to build the one-hot
    # eye matrix on device (iota + is_equal) that injects -BIG into the
    # diagonal score entries via one accumulating matmul
    rowcol = nc.dram_tensor("rowcol", [ROWS, 1], f32, kind="ExternalInput")
    negbig_eye = nc.dram_tensor("negbig_eye", [H, H], f32, kind="ExternalInput")

    bf16 = mybir.dt.bfloat16
    if mode in ("v7", "v8"):
        # v7 = v6 + row offload: the last K_OFF rows' scores are computed
        # off-PE in [j,h] layout (Pool adds B''+A''bcast, DVE does fused
        # relu*sgn+accum), freeing ~512ns of PE stream per row.
        koff = (V8_XD + V8_XA) if mode == "v8" else K_OFF
        s_out = nc.dram_tensor("s_out", [ROWS, N], bf16, kind="ExternalOutput")
        soff_out = nc.dram_tensor("soff_out", [H, 8 * koff], bf16,
                                  kind="ExternalOutput")
        w_out = m_out = None
        # [j,h]-path host-precomputed operands
        w1aw_t = nc.dram_tensor("w1aw_t", [H, H], f32, kind="ExternalInput")
        w1bw_t = nc.dram_tensor("w1bw_t", [H, H], f32, kind="ExternalInput")
        b1w_row = nc.dram_tensor("b1w_row", [1, H], f32, kind="ExternalInput")
        ones_row = nc.dram_tensor("ones_row", [1, ROWS], f32, kind="ExternalInput")
        sgn_bc = nc.dram_tensor("sgn_bc", [H, H], f32, kind="ExternalInput")
    elif mode.startswith("v6"):
        # v6: device ships raw scores+b2 as bf16; host does sigmoid, mask
        # (sign of bf16 is exact, so mask == f32 mask when b2==0 path is
        # biased on-device) and diagonal zeroing. Kills the device-side
        # sigmoid/is_gt/mask-DMA and the -1e30 eye-init matmuls.
        s_out = nc.dram_tensor("s_out", [ROWS, N], bf16, kind="ExternalOutput")
        w_out = m_out = None
    else:
        # weights leave the core as bf16 (halves the tail DMA); host upcasts.
        # Adds ~1e-3 rel err on weights vs the 2e-2 gate.
        w_out = nc.dram_tensor("w_out", [ROWS, N], bf16, kind="ExternalOutput")
        m_out = nc.dram_tensor("m_out", [ROWS, N], u8, kind="ExternalOutput")

    with TileContext(nc) as tc:
        with (
            tc.tile_pool(name="const", bufs=1) as cp,
            tc.tile_pool(name="xp", bufs=14) as xp,
            tc.tile_pool(name="yp8", bufs=14) as yp8,
            tc.tile_pool(name="pp", bufs=1, space="PSUM") as pp,
        ):
            emba_s = cp.tile([H, 512], f32, tag="emba")
            nc.sync.dma_start(out=emba_s[:], in_=emb_t[:, 0:512])
            embb_s = cp.tile([H, 512], f32, tag="embb")
            nc.sync.dma_start(out=embb_s[:], in_=emb_t[:, 512:1024])
            embr_s = cp.tile([H, ROWS], f32, tag="embr")
            nc.sync.dma_start(out=embr_s[:], in_=emb_rows_t[:])
            w1a_s = cp.tile([H, H], f32, tag="w1a")
            nc.sync.dma_start(out=w1a_s[:], in_=w1a_t[:])
            w1b_s = cp.tile([H, H], f32, tag="w1b")
            nc.sync.dma_start(out=w1b_s[:], in_=w1b_t[:])
            b1_s = cp.tile([H, 1], f32, tag="b1")
            nc.sync.dma_start(out=b1_s[:], in_=b1_col[:])
            z_s = cp.tile([H, 2 * H], f32, tag="z")
            nc.sync.dma_start(out=z_s[:], in_=zbuf[:])
            b2_s = cp.tile([H, 1], f32, tag="b2")
            nc.sync.dma_start(out=b2_s[:], in_=b2_col[:])
            if not mode.startswith(("v6", "v7")):
                nb2_s = cp.tile([H, 1], f32, tag="nb2")
                nc.sync.dma_start(out=nb2_s[:], in_=negb2_col[:])
                rc_s = cp.tile([ROWS, 1], f32, tag="rc")
                nc.sync.dma_start(out=rc_s[:], in_=rowcol[:])
                nbe_s = cp.tile([H, H], f32, tag="nbe")
                nc.sync.dma_start(out=nbe_s[:], in_=negbig_eye[:])

            # round f32r constants once (fp16 stationary for the fp16
            # moving-data variants: matmul can't mix 32/16-bit inputs)
            zdt = mybir.dt.float16 if mode in ("v6h", "v6hd", "v8") else f32r
            zr_s = cp.tile([H, 2 * H], zdt, tag="zr")
            nc.vector.tensor_copy(zr_s[:], z_s[:])
            if not mode.startswith(("v6", "v7")):
                nber_s = cp.tile([H, H], f32r, tag="nber")
                nc.vector.tensor_copy(nber_s[:], nbe_s[:])
            # f32r copies of emb / W1 halves: lets BT/CT run as 1-cycle/col
            # f32r matmuls instead of 4-cycle/col f32 (prologue-only cost)
            embar_s = cp.tile([H, 512], f32r, tag="embar")
            nc.vector.tensor_copy(embar_s[:], emba_s[:])
            embbr_s = cp.tile([H, 512], f32r, tag="embbr")
            nc.vector.tensor_copy(embbr_s[:], embb_s[:])
            embrr_s = cp.tile([H, ROWS], f32r, tag="embrr")
            nc.vector.tensor_copy(embrr_s[:], embr_s[:])
            w1ar_s = cp.tile([H, H], f32r, tag="w1ar")
            nc.vector.tensor_copy(w1ar_s[:], w1a_s[:])
            w1br_s = cp.tile([H, H], f32r, tag="w1br")
            nc.vector.tensor_copy(w1br_s[:], w1b_s[:])

            if not mode.startswith(("v6", "v7")):
                # build the one-hot eye matrix on device: eyr[k, j] = (j == rowcol[k])
                it_s = cp.tile([ROWS, N], f32, tag="it")
                nc.gpsimd.iota(it_s[:], pattern=[[1, N]], base=0,
                               channel_multiplier=0,
                               allow_small_or_imprecise_dtypes=True)
                eyr_s = cp.tile([ROWS, N], f32r, tag="eyr")
                nc.vector.tensor_scalar(
                    out=eyr_s[:],
                    in0=it_s[:],
                    scalar1=rc_s[:],
                    scalar2=None,
                    op0=mybir.AluOpType.is_equal,
                )

            if mode in ("v7", "v8"):
                # [j,h]-path constants
                w1aw_s = cp.tile([H, H], f32, tag="w1aw")
                nc.sync.dma_start(out=w1aw_s[:], in_=w1aw_t[:])
                w1bw_s = cp.tile([H, H], f32, tag="w1bw")
                nc.sync.dma_start(out=w1bw_s[:], in_=w1bw_t[:])
                b1w_s = cp.tile([1, H], f32, tag="b1w")
                nc.sync.dma_start(out=b1w_s[:], in_=b1w_row[:])
                ones_s0 = cp.tile([1, ROWS], f32, tag="ones0")
                nc.sync.dma_start(out=ones_s0[:], in_=ones_row[:])
                sgn_s = cp.tile([H, H], f32, tag="sgn")
                nc.sync.dma_start(out=sgn_s[:], in_=sgn_bc[:])
                w1awr_s = cp.tile([H, H], f32r, tag="w1awr")
                nc.vector.tensor_copy(w1awr_s[:], w1aw_s[:])
                w1bwr_s = cp.tile([H, H], f32r, tag="w1bwr")
                nc.vector.tensor_copy(w1bwr_s[:], w1bw_s[:])
                b1wr_s = cp.tile([1, H], f32r, tag="b1wr")
                nc.vector.tensor_copy(b1wr_s[:], b1w_s[:])
                ones_s = cp.tile([1, ROWS], f32r, tag="ones")
                nc.vector.tensor_copy(ones_s[:], ones_s0[:])

            # warm the PE HAM (clock gate) with dummy f32 matmuls while the
            # large input DMAs land, so prep + early main-loop matmuls run at
            # 2.4 GHz instead of the cold 1.2 GHz
            warm_ps = pp.tile([H, 128], f32, tag="warmp")
            for _w in range(12):
                nc.tensor.matmul(
                    warm_ps[:], w1a_s[:], w1a_s[:], start=True, stop=True
                )

            # force the sigmoid ACT table set to load during prep, so the
            # epilogue sigmoid doesn't pay a ~2.7us mid-kernel table swap
            # (relu/identity are filler entries in every set); reading
            # warm_ps also keeps the warm matmuls alive through DCE.
            # v6 has no device sigmoid: Identity is enough to defeat DCE.
            warm_s = cp.tile([H, 1], f32, tag="warm")
            nc.scalar.activation(
                warm_s[:], warm_ps[:, 0:1],
                mybir.ActivationFunctionType.Identity if mode.startswith(("v6", "v7"))
                else mybir.ActivationFunctionType.Sigmoid
            )

            if mode == "v8":
                # fp16 everywhere + row offload: all gens on DVE (2x mode),
                # XD rows reduced on DVE (relu*sgn + inner-axis reduce),
                # XA rows reduced on ACT (contiguous sign groups via host
                # h-permutation), adds on DVE.
                fp16 = mybir.dt.float16
                K_TAIL = 8
                KV8 = V8_XD + V8_XA
                NPE = ROWS - KV8
                PPOS, ZZ = _V8_PZ[0], _V8_PZ[1]
                NEG0 = PPOS + ZZ  # first negative-sign h index
                ct_ps = pp.tile([H, ROWS], f32, tag="ctp")
                ct_s = cp.tile([H, ROWS], f32, tag="ct")
                bt_ps = pp.tile([H, N], f32, tag="btp")
                bt_s = cp.tile([H, N], fp16, tag="bt")
                sc_ps = pp.tile([ROWS, N], f32, tag="scores")
                sb_s = cp.tile([ROWS, N], bf16, tag="sout")
                ct2_ps = pp.tile([ROWS, H], f32, tag="ct2p")
                ct2_s = cp.tile([ROWS, H], f32r, tag="ct2")
                bq_ps = bt_ps  # prep-only lifetimes: share the 2 PSUM banks
                bq_s = cp.tile([H, 8 * H], fp16, tag="bq")
                # 4 abc slots in one PSUM bank
                abc_ps = pp.tile([ROWS, 4 * H], f32, tag="abcp")
                abc_sb = [cp.tile([ROWS, H], fp16, tag=f"abs{u}",
                                  name=f"abs{u}") for u in range(KV8)]
                acc_s = cp.tile([H, 8 * KV8], f32, tag="acc")
                apn_s = cp.tile([H, 16 * KV8], f32, tag="apn")
                abf_s = cp.tile([H, 8 * KV8], bf16, tag="abf")
                junk_s = cp.tile([H, H], fp16, tag="junk")
                sgnh_s = cp.tile([H, H], fp16, tag="sgnh")
                nc.vector.tensor_copy(sgnh_s[:], sgn_s[:])
                ct2pk = cp.tile([1, KV8 * H], f32r, tag="ct2pk")

                def _rep8(ap):
                    lay = [list(d) for d in ap.ap]
                    return bass.AP(ap.tensor, ap.offset,
                                   [lay[0], [0, 8]] + lay[1:])

                def _blocked(ap):
                    # [128, 1024] viewed as [128, 8, 128] for inner reduce
                    lay = [list(d) for d in ap.ap]
                    return bass.AP(ap.tensor, ap.offset,
                                   [lay[0], [128, 8], [1, 128]])

                def prep():
                    nc.tensor.matmul(ct_ps[:], w1ar_s[:], embrr_s[:],
                                     start=True, stop=True)
                    nc.scalar.activation(
                        ct_s[:], ct_ps[:],
                        mybir.ActivationFunctionType.Identity, bias=b1_s[:])
                    nc.tensor.matmul(bt_ps[:, 0:512], w1br_s[:], embar_s[:],
                                     start=True, stop=True)
                    nc.tensor.matmul(bt_ps[:, 512:1024], w1br_s[:], embbr_s[:],
                                     start=True, stop=True)
                    nc.vector.tensor_copy(bt_s[:, 0:512], bt_ps[:, 0:512])
                    nc.vector.tensor_copy(bt_s[:, 512:1024], bt_ps[:, 512:1024])
                    nc.tensor.matmul(ct2_ps[:], embrr_s[:], w1awr_s[:],
                                     start=True, stop=False)
                    nc.tensor.matmul(ct2_ps[:], ones_s[:], b1wr_s[:],
                                     start=False, stop=True)
                    nc.scalar.activation(ct2_s[:], ct2_ps[:],
                                         mybir.ActivationFunctionType.Identity)
                    nc.sync.dma_start(out=ct2pk[:], in_=ct2_s[NPE:ROWS, :])
                    for bb in range(8):
                        lhsT = (embar_s[:, 128 * bb : 128 * bb + 128] if bb < 4
                                else embbr_s[:, 128 * (bb - 4) : 128 * (bb - 3)])
                        nc.tensor.matmul(
                            bq_ps[:, 128 * bb : 128 * bb + 128], lhsT,
                            w1bwr_s[:], start=True, stop=True)
                    nc.scalar.activation(bq_s[:, 0:512], bq_ps[:, 0:512],
                                         mybir.ActivationFunctionType.Identity)
                    nc.scalar.activation(bq_s[:, 512:1024], bq_ps[:, 512:1024],
                                         mybir.ActivationFunctionType.Identity)

                def abc_build(r):
                    sl = abc_ps[:, H * (r % 4) : H * (r % 4 + 1)]
                    nc.tensor.matmul(sl, ones_s[:],
                                     ct2pk[0:1, H * r : H * (r + 1)],
                                     start=True, stop=True)
                    nc.scalar.activation(abc_sb[r][:], sl,
                                         mybir.ActivationFunctionType.Identity)

                def add_row(r, yp_):
                    y = yp_.tile([H, N], fp16, tag="y", name="y")
                    nc.vector.scalar_tensor_tensor(
                        out=y[:], in0=bq_s[:], scalar=0.0,
                        op0=mybir.AluOpType.add, op1=mybir.AluOpType.add,
                        in1=_rep8(abc_sb[r][:]))
                    return y

                def dve_reduce(r, y):
                    y2 = xp.tile([H, N], fp16, tag="y2")
                    nc.vector.scalar_tensor_tensor(
                        out=y2[:], in0=y[:], scalar=0.0,
                        op0=mybir.AluOpType.max, op1=mybir.AluOpType.mult,
                        in1=_rep8(sgnh_s[:, 0:H]))
                    nc.vector.tensor_reduce(
                        out=acc_s[:, 8 * r : 8 * r + 8], in_=_blocked(y2[:]),
                        axis=mybir.AxisListType.X, op=mybir.AluOpType.add)

                def act_reduce(r, y):
                    for bb in range(8):
                        base = 128 * bb
                        nc.scalar.activation(
                            junk_s[:, 0:PPOS], y[:, base : base + PPOS],
                            mybir.ActivationFunctionType.Relu,
                            accum_out=apn_s[:, 16 * r + bb : 16 * r + bb + 1])
                        nc.scalar.activation(
                            junk_s[:, 0 : H - NEG0], y[:, base + NEG0 : base + H],
                            mybir.ActivationFunctionType.Relu,
                            accum_out=apn_s[:, 16 * r + 8 + bb
                                            : 16 * r + 8 + bb + 1])

                def pn_sub(r):
                    nc.vector.tensor_tensor(
                        out=acc_s[:, 8 * r : 8 * r + 8],
                        in0=apn_s[:, 16 * r : 16 * r + 8],
                        in1=apn_s[:, 16 * r + 8 : 16 * r + 16],
                        op=mybir.AluOpType.subtract)

                def body():
                    pend_h1 = []
                    acts = {}
                    for r in range(KV8):
                        # abc builds front-loaded (one per early row) so the
                        # ACT copies queue ahead of every ACT reduce batch;
                        # pos-neg subs all at the end (they wait on ACT)
                        acts.setdefault(r, []).append(("abc", r))
                        t = 2 + (r * (NPE - 12)) // KV8
                        acts.setdefault(t + 2, []).append(("add", r))
                        acts.setdefault(t + 3, []).append(("red", r))
                    ys = {}
                    for i in range(NPE):
                        x = xp.tile([H, N], fp16, tag="x")
                        nc.vector.tensor_scalar(
                            out=x[:], in0=bt_s[:],
                            scalar1=ct_s[:, i : i + 1], scalar2=0.0,
                            op0=mybir.AluOpType.add,
                            op1=mybir.AluOpType.max)
                        lhsT = zr_s[:, H - i : 2 * H - i]
                        nc.tensor.matmul(
                            sc_ps[:, 0:512], lhsT, x[:, 0:512],
                            start=(i == 0), stop=(i == NPE - 1),
                            skip_group_check=True)
                        if i < NPE - K_TAIL:
                            nc.tensor.matmul(
                                sc_ps[:, 512:1024], lhsT, x[:, 512:1024],
                                start=(i == 0), stop=False,
                                skip_group_check=True)
                        else:
                            pend_h1.append((i, x))
                        for kind, r in acts.get(i, ()):
                            if kind == "abc":
                                abc_build(r)
                            elif kind == "add":
                                ys[r] = add_row(r, yp8)
                            elif kind == "red":
                                if r < V8_XD:
                                    dve_reduce(r, ys[r])
                                else:
                                    act_reduce(r, ys[r])
                    for r in range(V8_XD, KV8):
                        pn_sub(r)
                    nc.scalar.activation(
                        sb_s[:, 0:512], sc_ps[:, 0:512],
                        mybir.ActivationFunctionType.Identity, bias=b2_s[:])
                    nc.sync.dma_start(out=s_out[:, 0:512], in_=sb_s[:, 0:512])
                    for i, x in pend_h1:
                        lhsT = zr_s[:, H - i : 2 * H - i]
                        nc.tensor.matmul(
                            sc_ps[:, 512:1024], lhsT, x[:, 512:1024],
                            start=False, stop=(i == NPE - 1),
                            skip_group_check=True)
                    nc.vector.tensor_scalar(
                        out=sb_s[:, 512:1024], in0=sc_ps[:, 512:1024],
                        scalar1=b2_s[:], scalar2=None,
                        op0=mybir.AluOpType.add)
                    nc.sync.dma_start(out=s_out[:, 512:1024],
                                      in_=sb_s[:, 512:1024])
                    nc.vector.tensor_scalar(
                        out=abf_s[:], in0=acc_s[:], scalar1=b2_s[:],
                        scalar2=None, op0=mybir.AluOpType.add)
                    nc.sync.dma_start(out=soff_out[:], in_=abf_s[:])
                    prep()

                prep()
            elif mode == "v7":
                # v6 structure + off-PE computation of the last K_OFF rows
                K_TAIL = 8
                NPE = ROWS - K_OFF
                ct_ps = pp.tile([H, ROWS], f32, tag="ctp")
                ct_s = cp.tile([H, ROWS], f32, tag="ct")
                bt_ps = pp.tile([H, N], f32, tag="btp")
                bt_s = cp.tile([H, N], f32, tag="bt")
                sc_ps = pp.tile([ROWS, N], f32, tag="scores")
                sb_s = cp.tile([ROWS, N], bf16, tag="sout")
                ct2_ps = pp.tile([ROWS, H], f32, tag="ct2p")
                ct2_s = cp.tile([ROWS, H], f32r, tag="ct2")
                bq_ps = bt_ps  # prep-only lifetimes: share the 2 PSUM banks
                bq_s = cp.tile([H, 8 * H], f32, tag="bq")
                abc_ps = [pp.tile([ROWS, H], f32, tag="abcp", name="abcp")] * 2
                abc_sb = [cp.tile([ROWS, H], f32, tag=f"abcs{u}",
                                  name=f"abcs{u}") for u in range(2)]
                acc_s = cp.tile([H, 8 * K_OFF], f32, tag="acc")
                abf_s = cp.tile([H, 8 * K_OFF], bf16, tag="abf")
                junk_s = cp.tile([H, H], f32, tag="junk")
                # PE moving APs must start at partition 0/32/64: pack the
                # K_OFF ct2 rows into partition 0's free dim via DMA
                ct2pk = cp.tile([1, K_OFF * H], f32r, tag="ct2pk")

                def _rep8(ap):
                    a = ap.copy()
                    a.ap = [a.ap[0], [0, 8]] + a.ap[1:]
                    return a

                def prep():
                    nc.tensor.matmul(ct_ps[:], w1ar_s[:], embrr_s[:],
                                     start=True, stop=True)
                    nc.scalar.activation(
                        ct_s[:], ct_ps[:],
                        mybir.ActivationFunctionType.Identity, bias=b1_s[:])
                    nc.tensor.matmul(bt_ps[:, 0:512], w1br_s[:], embar_s[:],
                                     start=True, stop=True)
                    nc.tensor.matmul(bt_ps[:, 512:1024], w1br_s[:], embbr_s[:],
                                     start=True, stop=True)
                    nc.vector.tensor_copy(bt_s[:, 0:512], bt_ps[:, 0:512])
                    nc.vector.tensor_copy(bt_s[:, 512:1024], bt_ps[:, 512:1024])
                    # ct2T[i, h] = |W2_h|*(a_i[h] + b1[h])
                    nc.tensor.matmul(ct2_ps[:], embrr_s[:], w1awr_s[:],
                                     start=True, stop=False)
                    nc.tensor.matmul(ct2_ps[:], ones_s[:], b1wr_s[:],
                                     start=False, stop=True)
                    nc.scalar.activation(ct2_s[:], ct2_ps[:],
                                         mybir.ActivationFunctionType.Identity)
                    nc.sync.dma_start(out=ct2pk[:], in_=ct2_s[NPE:ROWS, :])
                    # B''[j, h] = |W2_h| * (emb @ W1b.T)[j, h], 8 j-blocks
                    for bb in range(8):
                        lhsT = (embar_s[:, 128 * bb : 128 * bb + 128] if bb < 4
                                else embbr_s[:, 128 * (bb - 4) : 128 * (bb - 3)])
                        nc.tensor.matmul(
                            bq_ps[:, 128 * bb : 128 * bb + 128], lhsT,
                            w1bwr_s[:], start=True, stop=True)
                    nc.scalar.activation(bq_s[:, 0:512], bq_ps[:, 0:512],
                                         mybir.ActivationFunctionType.Identity)
                    nc.scalar.activation(bq_s[:, 512:1024], bq_ps[:, 512:1024],
                                         mybir.ActivationFunctionType.Identity)

                def off_row(r):
                    # scores for local row i = NPE + r, computed off-PE
                    u = r % 2
                    nc.tensor.matmul(abc_ps[u][:], ones_s[:],
                                     ct2pk[0:1, H * r : H * (r + 1)],
                                     start=True, stop=True)
                    nc.scalar.activation(abc_sb[u][:], abc_ps[u][:],
                                         mybir.ActivationFunctionType.Identity)
                    y = xp.tile([H, N], f32, tag="y")
                    nc.gpsimd.tensor_tensor(out=y[:], in0=bq_s[:],
                                            in1=_rep8(abc_sb[u][:]),
                                            op=mybir.AluOpType.add)
                    for bb in range(8):
                        nc.vector.scalar_tensor_tensor(
                            out=junk_s[:], in0=y[:, 128 * bb : 128 * bb + 128],
                            scalar=0.0, op0=mybir.AluOpType.max,
                            op1=mybir.AluOpType.mult, in1=sgn_s[:],
                            accum_out=acc_s[:, 8 * r + bb : 8 * r + bb + 1])

                def body():
                    pend_h1 = []
                    trigger = {max(0, ((r + 1) * NPE) // K_OFF - 4): r
                               for r in range(K_OFF)}
                    for i in range(NPE):
                        x = xp.tile([H, N], f32r, tag="x")
                        if i % 12 in ACT_GEN:
                            nc.scalar.activation(
                                x[:], bt_s[:],
                                mybir.ActivationFunctionType.Relu,
                                bias=ct_s[:, i : i + 1])
                        else:
                            nc.vector.tensor_scalar(
                                out=x[:], in0=bt_s[:],
                                scalar1=ct_s[:, i : i + 1], scalar2=0.0,
                                op0=mybir.AluOpType.add,
                                op1=mybir.AluOpType.max)
                        lhsT = zr_s[:, H - i : 2 * H - i]
                        nc.tensor.matmul(
                            sc_ps[:, 0:512], lhsT, x[:, 0:512],
                            start=(i == 0), stop=(i == NPE - 1),
                            skip_group_check=True)
                        if i < NPE - K_TAIL:
                            nc.tensor.matmul(
                                sc_ps[:, 512:1024], lhsT, x[:, 512:1024],
                                start=(i == 0), stop=False,
                                skip_group_check=True)
                        else:
                            pend_h1.append((i, x))
                        if i in trigger:
                            off_row(trigger[i])
                    nc.scalar.activation(
                        sb_s[:, 0:512], sc_ps[:, 0:512],
                        mybir.ActivationFunctionType.Identity, bias=b2_s[:])
                    nc.sync.dma_start(out=s_out[:, 0:512], in_=sb_s[:, 0:512])
                    for i, x in pend_h1:
                        lhsT = zr_s[:, H - i : 2 * H - i]
                        nc.tensor.matmul(
                            sc_ps[:, 512:1024], lhsT, x[:, 512:1024],
                            start=False, stop=(i == NPE - 1),
                            skip_group_check=True)
                    nc.vector.tensor_scalar(
                        out=sb_s[:, 512:1024], in0=sc_ps[:, 512:1024],
                        scalar1=b2_s[:], scalar2=None,
                        op0=mybir.AluOpType.add)
                    nc.sync.dma_start(out=s_out[:, 512:1024],
                                      in_=sb_s[:, 512:1024])
                    nc.vector.tensor_scalar(
                        out=abf_s[:], in0=acc_s[:], scalar1=b2_s[:],
                        scalar2=None, op0=mybir.AluOpType.add)
                    nc.sync.dma_start(out=soff_out[:], in_=abf_s[:])
                    prep()

                prep()
            elif mode in ("v6", "v6pool", "v6h", "v6hd"):
                # software-pipelined like v4, but: no eye-init (host zeroes
                # the diagonal), raw bf16 scores out (host sigmoid/mask),
                # and the last K_TAIL rows issue all their half-0 matmuls
                # before the half-1 burst so the half-0 convert+DMA hides
                # under ~2us of remaining PE work.
                K_TAIL = 8
                # v6h: bt/X in fp16 — same PE column rate, DVE gens in 2x
                # mode, ~f32r-level precision (10-bit mantissa)
                xdt = mybir.dt.float16 if mode in ("v6h", "v6hd") else f32r
                btdt = mybir.dt.float16 if mode in ("v6h", "v6hd") else f32
                ct_ps = pp.tile([H, ROWS], f32, tag="ctp")
                ct_s = cp.tile([H, ROWS], f32, tag="ct")
                bt_ps = pp.tile([H, N], f32, tag="btp")
                bt_s = cp.tile([H, N], btdt, tag="bt")
                sc_ps = pp.tile([ROWS, N], f32, tag="scores")
                sb_s = cp.tile([ROWS, N], bf16, tag="sout")

                def prep():
                    nc.tensor.matmul(ct_ps[:], w1ar_s[:], embrr_s[:],
                                     start=True, stop=True)
                    nc.scalar.activation(
                        ct_s[:], ct_ps[:],
                        mybir.ActivationFunctionType.Identity, bias=b1_s[:])
                    nc.tensor.matmul(bt_ps[:, 0:512], w1br_s[:], embar_s[:],
                                     start=True, stop=True)
                    nc.tensor.matmul(bt_ps[:, 512:1024], w1br_s[:], embbr_s[:],
                                     start=True, stop=True)
                    nc.vector.tensor_copy(bt_s[:, 0:512], bt_ps[:, 0:512])
                    nc.vector.tensor_copy(bt_s[:, 512:1024], bt_ps[:, 512:1024])

                def _rep_ap(ap, reps):
                    # stride-0 free-dim repeat: [p] + [0, reps] + [inner]
                    a = ap.copy()
                    a.ap = [a.ap[0], [0, reps]] + a.ap[1:]
                    return a

                if mode == "v6pool":
                    # throughput probe: does the idle Pool engine absorb
                    # ~12 rows' worth of [j,h]-style add + relu*sgn+accum
                    # work without moving the critical path?
                    KPROBE = int(__import__("os").environ.get("KPROBE", "8"))
                    ydum = cp.tile([H, N], f32, tag="ydum")
                    acc = cp.tile([H, KPROBE], f32, tag="acc")
                    in0_rep = _rep_ap(emba_s[:, 0:512], 2)
                    in1_rep = _rep_ap(w1a_s[:], 8)

                def pool_probe():
                    for r in range(KPROBE):
                        nc.gpsimd.tensor_tensor(
                            out=ydum[:], in0=in0_rep, in1=in1_rep,
                            op=mybir.AluOpType.add)

                def body():
                    if mode == "v6pool":
                        pool_probe()
                    pend_h1 = []
                    for i in range(ROWS):
                        x = xp.tile([H, N], xdt, tag="x")
                        if i % 3 == 1 and mode != "v6hd":
                            nc.scalar.activation(
                                x[:], bt_s[:],
                                mybir.ActivationFunctionType.Relu,
                                bias=ct_s[:, i : i + 1])
                        else:
                            nc.vector.tensor_scalar(
                                out=x[:], in0=bt_s[:],
                                scalar1=ct_s[:, i : i + 1], scalar2=0.0,
                                op0=mybir.AluOpType.add,
                                op1=mybir.AluOpType.max)
                        lhsT = zr_s[:, H - i : 2 * H - i]
                        nc.tensor.matmul(
                            sc_ps[:, 0:512], lhsT, x[:, 0:512],
                            start=(i == 0), stop=(i == ROWS - 1),
                            skip_group_check=True)
                        if i < ROWS - K_TAIL:
                            nc.tensor.matmul(
                                sc_ps[:, 512:1024], lhsT, x[:, 512:1024],
                                start=(i == 0), stop=False,
                                skip_group_check=True)
                        else:
                            pend_h1.append((i, x))
                    # half-0 finished: convert+DMA it on ACT while the PE
                    # drains the deferred half-1 matmuls
                    nc.scalar.activation(
                        sb_s[:, 0:512], sc_ps[:, 0:512],
                        mybir.ActivationFunctionType.Identity, bias=b2_s[:])
                    nc.sync.dma_start(out=s_out[:, 0:512], in_=sb_s[:, 0:512])
                    for i, x in pend_h1:
                        lhsT = zr_s[:, H - i : 2 * H - i]
                        nc.tensor.matmul(
                            sc_ps[:, 512:1024], lhsT, x[:, 512:1024],
                            start=False, stop=(i == ROWS - 1),
                            skip_group_check=True)
                    # half-1 convert on DVE (ACT may still be busy with h0)
                    nc.vector.tensor_scalar(
                        out=sb_s[:, 512:1024], in0=sc_ps[:, 512:1024],
                        scalar1=b2_s[:], scalar2=None,
                        op0=mybir.AluOpType.add)
                    nc.sync.dma_start(out=s_out[:, 512:1024],
                                      in_=sb_s[:, 512:1024])
                    prep()

                prep()
            elif mode in ("full", "v4", "v4s", "v5", "v5s"):
                # software-pipelined: BT/CT/eye-init are rep-invariant, so
                # each body computes them for the NEXT rep during the DMA
                # tail; gens read the copies produced by the previous rep
                ct_ps = pp.tile([H, ROWS], f32, tag="ctp")
                ct_s = cp.tile([H, ROWS], f32, tag="ct")
                bt_ps = pp.tile([H, N], f32, tag="btp")
                bt_s = cp.tile([H, N], f32, tag="bt")
                sc_ps = pp.tile([ROWS, N], f32, tag="scores")
                sig_s = cp.tile([ROWS, N], mybir.dt.bfloat16, tag="sig")
                m_s = cp.tile([ROWS, N], u8, tag="m")

                def prep():
                    nc.tensor.matmul(ct_ps[:], w1ar_s[:], embrr_s[:],
                                     start=True, stop=True)
                    nc.scalar.activation(
                        ct_s[:], ct_ps[:],
                        mybir.ActivationFunctionType.Identity, bias=b1_s[:])
                    nc.tensor.matmul(bt_ps[:, 0:512], w1br_s[:], embar_s[:],
                                     start=True, stop=True)
                    nc.tensor.matmul(bt_ps[:, 512:1024], w1br_s[:], embbr_s[:],
                                     start=True, stop=True)
                    nc.vector.tensor_copy(bt_s[:, 0:512], bt_ps[:, 0:512])
                    nc.vector.tensor_copy(bt_s[:, 512:1024], bt_ps[:, 512:1024])
                    for h0 in (0, 512):
                        nc.tensor.matmul(
                            sc_ps[:, h0 : h0 + 512], nber_s[:],
                            eyr_s[:, h0 : h0 + 512],
                            start=True, stop=False, skip_group_check=True)

                def body():
                    for i in range(ROWS):
                        x = xp.tile([H, N], f32r, tag="x")
                        if i % 3 == 1:
                            nc.scalar.activation(
                                x[:], bt_s[:],
                                mybir.ActivationFunctionType.Relu,
                                bias=ct_s[:, i : i + 1])
                        else:
                            nc.vector.tensor_scalar(
                                out=x[:], in0=bt_s[:],
                                scalar1=ct_s[:, i : i + 1], scalar2=0.0,
                                op0=mybir.AluOpType.add,
                                op1=mybir.AluOpType.max)
                        lhsT = zr_s[:, H - i : 2 * H - i]
                        nc.tensor.matmul(
                            sc_ps[:, 0:512], lhsT, x[:, 0:512],
                            start=False, stop=(i == ROWS - 1),
                            skip_group_check=True)
                        nc.tensor.matmul(
                            sc_ps[:, 512:1024], lhsT, x[:, 512:1024],
                            start=False, stop=(i == ROWS - 1),
                            skip_group_check=True)
                    if mode not in ("v5", "v5s"):
                        for h0 in (0, 512):
                            nc.scalar.activation(
                                sig_s[:, h0 : h0 + 512], sc_ps[:, h0 : h0 + 512],
                                mybir.ActivationFunctionType.Sigmoid, bias=b2_s[:])
                            nc.sync.dma_start(out=w_out[:, h0 : h0 + 512],
                                              in_=sig_s[:, h0 : h0 + 512])
                        nc.vector.tensor_scalar(
                            out=m_s[:], in0=sc_ps[:], scalar1=nb2_s[:],
                            scalar2=None, op0=mybir.AluOpType.is_gt)
                        nc.sync.dma_start(out=m_out[:], in_=m_s[:])
                        prep()
                        return
                    # v5: engine-queue-aware ordering of tail + next-rep prep:
                    # DVE [gens, btcopies, is_gt], ACT [gens, sigmoids, ct],
                    # PE [MMs, BT, CT, eye] so next-rep inputs land earliest
                    for h0 in (0, 512):
                        nc.scalar.activation(
                            sig_s[:, h0 : h0 + 512], sc_ps[:, h0 : h0 + 512],
                            mybir.ActivationFunctionType.Sigmoid, bias=b2_s[:])
                        nc.sync.dma_start(out=w_out[:, h0 : h0 + 512],
                                          in_=sig_s[:, h0 : h0 + 512])
                    nc.tensor.matmul(bt_ps[:, 0:512], w1br_s[:], embar_s[:],
                                     start=True, stop=True)
                    nc.tensor.matmul(bt_ps[:, 512:1024], w1br_s[:], embbr_s[:],
                                     start=True, stop=True)
                    nc.vector.tensor_copy(bt_s[:, 0:512], bt_ps[:, 0:512])
                    nc.vector.tensor_copy(bt_s[:, 512:1024], bt_ps[:, 512:1024])
                    nc.tensor.matmul(ct_ps[:], w1ar_s[:], embrr_s[:],
                                     start=True, stop=True)
                    nc.vector.tensor_scalar(
                        out=m_s[:], in0=sc_ps[:], scalar1=nb2_s[:],
                        scalar2=None, op0=mybir.AluOpType.is_gt)
                    nc.sync.dma_start(out=m_out[:], in_=m_s[:])
                    nc.scalar.activation(
                        ct_s[:], ct_ps[:],
                        mybir.ActivationFunctionType.Identity, bias=b1_s[:])
                    for h0 in (0, 512):
                        nc.tensor.matmul(
                            sc_ps[:, h0 : h0 + 512], nber_s[:],
                            eyr_s[:, h0 : h0 + 512],
                            start=True, stop=False, skip_group_check=True)

                prep()
            elif mode == "v3":
                def body():
                    _body_v3(nc, tc, cp, xp, pp, mybir, f32, f32r, u8,
                             embar_s, embbr_s, embrr_s, w1ar_s, w1br_s, b1_s,
                             zr_s, b2_s, nb2_s, eyr_s, nber_s, w_out, m_out)
            elif mode in ("v2psum", "v2sb"):
                def body():
                    _body_v2(nc, tc, cp, xp, pp, mybir, f32, f32r, u8,
                             embar_s, embbr_s, embrr_s, w1ar_s, w1br_s, b1_s,
                             zr_s, b2_s, nb2_s, eyr_s, nber_s, w_out, m_out,
                             act_src_psum=(mode == "v2psum"))
            else:
                def body():
                    _body_once(nc, tc, cp, xp, pp, mybir, f32, f32r, u8,
                               (emba_s, embb_s), embr_s, w1a_s, w1b_s, b1_s,
                               zr_s, b2_s, nb2_s, eyr_s, nber_s, w_out, m_out,
                               mode)

            if loop_reps > 1:
                # staggered_reset drops the per-iteration all-engine barrier
                # (rolling per-stage sem resets instead), letting engines flow
                # into the next rep while others drain the tail
                with tc.For_i(0, loop_reps, 1,
                              staggered_reset=(mode in ("v4s", "v5s"))):
                    body()
            else:
                for _rep in range(reps):
                    body()

    _split_multiwaits(nc)
    return nc


def _body_v2(nc, tc, cp, xp, pp, mybir, f32, f32r, u8,
             embar_s, embbr_s, embrr_s, w1ar_s, w1br_s, b1_s, zr_s, b2_s,
             nb2_s, eyr_s, nber_s, w_out, m_out, act_src_psum=True):
    """PE-rate-bound body: f32r BT/CT, ACT gens read bt straight from PSUM,
    DVE gens read an SBUF copy; pipelined epilogue halves."""
    # BT = W1b @ embT (f32r matmuls: 1 cyc/col instead of 4)
    bt_ps = pp.tile([H, N], f32, tag="btp")
    nc.tensor.matmul(bt_ps[:, 0:512], w1br_s[:], embar_s[:], start=True, stop=True)
    nc.tensor.matmul(bt_ps[:, 512:1024], w1br_s[:], embbr_s[:], start=True, stop=True)
    # CT = W1a @ embT_rows + b1
    ct_ps = pp.tile([H, ROWS], f32, tag="ctp")
    nc.tensor.matmul(ct_ps[:], w1ar_s[:], embrr_s[:], start=True, stop=True)
    ct_s = cp.tile([H, ROWS], f32, tag="ct")
    nc.scalar.activation(
        ct_s[:], ct_ps[:], mybir.ActivationFunctionType.Identity, bias=b1_s[:]
    )
    # SBUF copy of bt for the DVE generators (DVE from PSUM would drop to 1x)
    bt_s = cp.tile([H, N], f32, tag="bt")
    nc.vector.tensor_copy(bt_s[:], bt_ps[:])

    # scores PSUM, diagonal pre-initialized to -BIG
    sc_ps = pp.tile([ROWS, N], f32, tag="scores")
    for h0 in (0, 512):
        nc.tensor.matmul(
            sc_ps[:, h0 : h0 + 512], nber_s[:], eyr_s[:, h0 : h0 + 512],
            start=True, stop=False,
        )

    # main loop: ACT tiles early (ACT is ready before the bt SBUF copy lands),
    # then interleave so both engines stay fed; ACT reads bt from PSUM
    # ((N+172)/1.2 vs (N+352)/1.2 from SBUF)
    act_set = {0, 1}
    act_set.update(i for i in range(2, ROWS) if i % 3 == 2)
    for i in range(ROWS):
        x = xp.tile([H, N], f32r, tag="x")
        if i in act_set:
            nc.scalar.activation(
                x[:], bt_ps[:] if act_src_psum else bt_s[:],
                mybir.ActivationFunctionType.Relu,
                bias=ct_s[:, i : i + 1],
            )
        else:
            nc.vector.tensor_scalar(
                out=x[:], in0=bt_s[:],
                scalar1=ct_s[:, i : i + 1], scalar2=0.0,
                op0=mybir.AluOpType.add, op1=mybir.AluOpType.max,
            )
        lhsT = zr_s[:, H - i : 2 * H - i]
        nc.tensor.matmul(
            sc_ps[:, 0:512], lhsT, x[:, 0:512],
            start=False, stop=(i == ROWS - 1),
        )
        nc.tensor.matmul(
            sc_ps[:, 512:1024], lhsT, x[:, 512:1024],
            start=False, stop=(i == ROWS - 1),
        )

    # epilogue: halves so the first w_out DMA overlaps the second sigmoid
    sig_s = cp.tile([ROWS, N], mybir.dt.bfloat16, tag="sig")
    for h0 in (0, 512):
        nc.scalar.activation(
            sig_s[:, h0 : h0 + 512], sc_ps[:, h0 : h0 + 512],
            mybir.ActivationFunctionType.Sigmoid, bias=b2_s[:],
        )
        nc.sync.dma_start(out=w_out[:, h0 : h0 + 512], in_=sig_s[:, h0 : h0 + 512])
    m_s = cp.tile([ROWS, N], u8, tag="m")
    nc.vector.tensor_scalar(
        out=m_s[:], in0=sc_ps[:], scalar1=nb2_s[:], scalar2=None,
        op0=mybir.AluOpType.is_gt,
    )
    nc.sync.dma_start(out=m_out[:], in_=m_s[:])


def _body_v3(nc, tc, cp, xp, pp, mybir, f32, f32r, u8,
             embar_s, embbr_s, embrr_s, w1ar_s, w1br_s, b1_s, zr_s, b2_s,
             nb2_s, eyr_s, nber_s, w_out, m_out):
    """fullold dataflow (SBUF-src gens, split bt copies on DVE+ACT) with:
    CT-first head, f32r BT/CT matmuls, pipelined sigmoid/DMA tail."""
    # CT first so ct_s is ready before the first ACT generation
    ct_ps = pp.tile([H, ROWS], f32, tag="ctp")
    nc.tensor.matmul(ct_ps[:], w1ar_s[:], embrr_s[:], start=True, stop=True)
    ct_s = cp.tile([H, ROWS], f32, tag="ct")
    nc.scalar.activation(
        ct_s[:], ct_ps[:], mybir.ActivationFunctionType.Identity, bias=b1_s[:]
    )
    bt_ps = pp.tile([H, N], f32, tag="btp")
    nc.tensor.matmul(bt_ps[:, 0:512], w1br_s[:], embar_s[:], start=True, stop=True)
    nc.tensor.matmul(bt_ps[:, 512:1024], w1br_s[:], embbr_s[:], start=True, stop=True)
    bt_s = cp.tile([H, N], f32, tag="bt")
    nc.vector.tensor_copy(bt_s[:, 0:512], bt_ps[:, 0:512])
    nc.scalar.copy(bt_s[:, 512:1024], bt_ps[:, 512:1024])

    sc_ps = pp.tile([ROWS, N], f32, tag="scores")
    for h0 in (0, 512):
        nc.tensor.matmul(
            sc_ps[:, h0 : h0 + 512], nber_s[:], eyr_s[:, h0 : h0 + 512],
            start=True, stop=False,
        )

    for i in range(ROWS):
        x = xp.tile([H, N], f32r, tag="x")
        if i % 3 == 1:
            nc.scalar.activation(
                x[:], bt_s[:], mybir.ActivationFunctionType.Relu,
                bias=ct_s[:, i : i + 1],
            )
        else:
            nc.vector.tensor_scalar(
                out=x[:], in0=bt_s[:],
                scalar1=ct_s[:, i : i + 1], scalar2=0.0,
                op0=mybir.AluOpType.add, op1=mybir.AluOpType.max,
            )
        lhsT = zr_s[:, H - i : 2 * H - i]
        nc.tensor.matmul(
            sc_ps[:, 0:512], lhsT, x[:, 0:512],
            start=False, stop=(i == ROWS - 1),
        )
        nc.tensor.matmul(
            sc_ps[:, 512:1024], lhsT, x[:, 512:1024],
            start=False, stop=(i == ROWS - 1),
        )

    sig_s = cp.tile([ROWS, N], mybir.dt.bfloat16, tag="sig")
    for h0 in (0, 512):
        nc.scalar.activation(
            sig_s[:, h0 : h0 + 512], sc_ps[:, h0 : h0 + 512],
            mybir.ActivationFunctionType.Sigmoid, bias=b2_s[:],
        )
        nc.sync.dma_start(out=w_out[:, h0 : h0 + 512], in_=sig_s[:, h0 : h0 + 512])
    m_s = cp.tile([ROWS, N], u8, tag="m")
    nc.vector.tensor_scalar(
        out=m_s[:], in0=sc_ps[:], scalar1=nb2_s[:], scalar2=None,
        op0=mybir.AluOpType.is_gt,
    )
    nc.sync.dma_start(out=m_out[:], in_=m_s[:])


def _body_once(nc, tc, cp, xp, pp, mybir, f32, f32r, u8,
               embt_halves, embr_s, w1a_s, w1b_s, b1_s, zr_s, b2_s, nb2_s,
               eyr_s, nber_s, w_out, m_out, mode="full"):
    emba_s, embb_s = embt_halves
    if mode == "empty":
        return
    if True:
        if True:
            # BT = W1b @ embT  (f32, exact): psum half per matmul; each half
            # depends only on its own emb DMA, and the PSUM->SBUF copies run
            # on different engines so they overlap
            bt_ps = pp.tile([H, N], f32, tag="btp")
            nc.tensor.matmul(
                bt_ps[:, 0:512], w1b_s[:], emba_s[:], start=True, stop=True
            )
            nc.tensor.matmul(
                bt_ps[:, 512:1024], w1b_s[:], embb_s[:], start=True, stop=True
            )
            bt_s = cp.tile([H, N], f32, tag="bt")
            nc.vector.tensor_copy(bt_s[:, 0:512], bt_ps[:, 0:512])
            nc.scalar.copy(bt_s[:, 512:1024], bt_ps[:, 512:1024])

            # CT = W1a @ embT_rows + b1  (f32, exact)
            ct_ps = pp.tile([H, ROWS], f32, tag="ctp")
            nc.tensor.matmul(ct_ps[:], w1a_s[:], embr_s[:], start=True, stop=True)
            ct_s = cp.tile([H, ROWS], f32, tag="ct")
            nc.scalar.activation(
                ct_s[:], ct_ps[:], mybir.ActivationFunctionType.Identity, bias=b1_s[:]
            )

            # main loop: accumulate scores into PSUM [128 rows, 1024 cols]
            sc_ps = pp.tile([ROWS, N], f32, tag="scores")
            # initialize each scores bank with -BIG at the diagonal entries
            # (zeros elsewhere): out[k, j] = -BIG*eye[k, j]; keeps the
            # epilogue off the critical tail
            for h0 in (0, 512):
                nc.tensor.matmul(
                    sc_ps[:, h0 : h0 + 512],
                    nber_s[:],
                    eyr_s[:, h0 : h0 + 512],
                    start=True,
                    stop=False,
                )
            xfix = None
            if mode in ("nogen", "nogen_fixw", "nogen_w32", "nogen_1bank",
                        "nogen_fixw32", "nogen_b8", "nogen_256",
                        "nogen_noacc"):
                xfix = cp.tile([H, N], f32r, tag="xfix")
                nc.vector.tensor_copy(xfix[:, 0:256], zr_s[:])
            if mode == "nogen_noacc":
                # 256 fresh-write (start+stop) MMs into one PSUM region:
                # does dropping the accumulate read-modify-write raise the
                # column rate?
                lhsT = zr_s[:, 0:H]
                for i in range(2 * ROWS):
                    nc.tensor.matmul(
                        sc_ps[:, 0:512], lhsT, xfix[:, 0:512],
                        start=True, stop=True, skip_group_check=True,
                    )
                _epilogue(nc, cp, mybir, f32, u8, sc_ps, b2_s, nb2_s, w_out, m_out)
                return
            if mode == "nogen_256":
                # same total moving cols as nogen, but 512 MMs x 256 cols:
                # separates per-MM fixed overhead from cycle-rate
                lhsT = zr_s[:, 0:H]
                for i in range(2 * ROWS):
                    for c0 in (0, 256):
                        nc.tensor.matmul(
                            sc_ps[:, c0 : c0 + 256], lhsT, xfix[:, c0 : c0 + 256],
                            start=False, stop=(i == 2 * ROWS - 1 and c0 == 256),
                        )
                _epilogue(nc, cp, mybir, f32, u8, sc_ps, b2_s, nb2_s, w_out, m_out)
                return
            if mode == "nogen_bf16":
                # 256 MMs x 512 cols with bf16 moving + bf16 stationary:
                # tests whether the ~250ns/MM is f32r-specific or clock/overhead
                bf16 = mybir.dt.bfloat16
                xbf = cp.tile([H, N], bf16, tag="xbf")
                nc.vector.tensor_copy(xbf[:, 0:256], zr_s[:])
                zbf = cp.tile([H, H], bf16, tag="zbf")
                nc.vector.tensor_copy(zbf[:], zr_s[:, 0:H])
                for i in range(2 * ROWS):
                    nc.tensor.matmul(
                        sc_ps[:, 0:512], zbf[:], xbf[:, 0:512],
                        start=False, stop=(i == 2 * ROWS - 1),
                    )
                _epilogue(nc, cp, mybir, f32, u8, sc_ps, b2_s, nb2_s, w_out, m_out)
                return
            if mode == "nogen_1bank":
                # fixed 128-wide stationary, all MMs -> one PSUM bank
                lhsT = zr_s[:, 0:H]
                for i in range(2 * ROWS):
                    nc.tensor.matmul(
                        sc_ps[:, 0:512], lhsT, xfix[:, 0:512],
                        start=False, stop=(i == 2 * ROWS - 1),
                    )
                _epilogue(nc, cp, mybir, f32, u8, sc_ps, b2_s, nb2_s, w_out, m_out)
                return
            if mode == "nogen_fixw32":
                # fixed 32-wide stationary, all MMs -> one PSUM region
                lhsT = zr_s[:, 128:160]
                for i in range(2 * ROWS):
                    nc.tensor.matmul(
                        sc_ps[0:32, 0:512], lhsT, xfix[:, 0:512],
                        start=False, stop=(i == 2 * ROWS - 1),
                    )
                _epilogue(nc, cp, mybir, f32, u8, sc_ps, b2_s, nb2_s, w_out, m_out)
                return
            if mode == "nogen_b8":
                # sliding stationaries, banks switched every 8 rows
                for i0 in range(0, ROWS, 8):
                    for h0 in (0, 512):
                        for i in range(i0, i0 + 8):
                            lhsT = zr_s[:, H - i : 2 * H - i]
                            nc.tensor.matmul(
                                sc_ps[:, h0 : h0 + 512], lhsT, xfix[:, h0 : h0 + 512],
                                start=False,
                                stop=(i == ROWS - 1),
                            )
                _epilogue(nc, cp, mybir, f32, u8, sc_ps, b2_s, nb2_s, w_out, m_out)
                return
            if mode == "nogen_fixw":
                # PE-only, FIXED stationary: isolates LDWEIGHTS cost vs nogen
                lhsT = zr_s[:, 0:H]
                for i in range(ROWS):
                    nc.tensor.matmul(
                        sc_ps[:, 0:512], lhsT, xfix[:, 0:512],
                        start=False, stop=(i == ROWS - 1),
                    )
                    nc.tensor.matmul(
                        sc_ps[:, 512:1024], lhsT, xfix[:, 512:1024],
                        start=False, stop=(i == ROWS - 1),
                    )
                _epilogue(nc, cp, mybir, f32, u8, sc_ps, b2_s, nb2_s, w_out, m_out)
                return
            if mode == "nogen_w32":
                # PE-only, 32-wide sliding stationaries + tile_position groups
                for g in range(4):
                    for k in range(32):
                        lhsT = zr_s[:, H - k : H + 32 - k]
                        for h0 in (0, 512):
                            nc.tensor.matmul(
                                sc_ps[32 * g : 32 * g + 32, h0 : h0 + 512],
                                lhsT,
                                xfix[:, h0 : h0 + 512],
                                start=False,
                                stop=(k == 31),
                                tile_position=(0, 32 * g),
                            )
                _epilogue(nc, cp, mybir, f32, u8, sc_ps, b2_s, nb2_s, w_out, m_out)
                return
            if mode == "full2":
                # col-group tiled reduction: 32-wide stationaries, 4 strips
                for k in range(32):
                    for g in range(4):
                        i = 32 * g + k
                        x = xp.tile([H, N], f32r, tag="x")
                        if (i * 5) % 13 < 5:
                            nc.scalar.activation(
                                x[:],
                                bt_s[:],
                                mybir.ActivationFunctionType.Relu,
                                bias=ct_s[:, i : i + 1],
                            )
                        else:
                            nc.vector.tensor_scalar(
                                out=x[:],
                                in0=bt_s[:],
                                scalar1=ct_s[:, i : i + 1],
                                scalar2=0.0,
                                op0=mybir.AluOpType.add,
                                op1=mybir.AluOpType.max,
                            )
                        lhsT = zr_s[:, H - k : H + 32 - k]
                        for h0 in (0, 512):
                            nc.tensor.matmul(
                                sc_ps[32 * g : 32 * g + 32, h0 : h0 + 512],
                                lhsT,
                                x[:, h0 : h0 + 512],
                                start=(k == 0),
                                stop=(k == 31),
                                tile_position=(0, 32 * g),
                            )
                _epilogue(nc, cp, mybir, f32, u8, sc_ps, b2_s, nb2_s, w_out, m_out)
                return

            for i in range(ROWS):
                if mode != "nogen":
                    x = xp.tile([H, N], f32r, tag="x")
                    if mode == "actgen" or (mode != "dvegen" and i % 3 == 1):
                        # ACT path: relu(in + bias), ~1147ns
                        nc.scalar.activation(
                            x[:],
                            bt_s[:],
                            mybir.ActivationFunctionType.Relu,
                            bias=ct_s[:, i : i + 1],
                        )
                    else:
                        # DVE path: (in + c_i) then max(.,0), ~720ns
                        nc.vector.tensor_scalar(
                            out=x[:],
                            in0=bt_s[:],
                            scalar1=ct_s[:, i : i + 1],
                            scalar2=0.0,
                            op0=mybir.AluOpType.add,
                            op1=mybir.AluOpType.max,
                        )
                else:
                    x = xfix
                if mode == "nomm":
                    continue
                lhsT = zr_s[:, H - i : 2 * H - i]
                nc.tensor.matmul(
                    sc_ps[:, 0:512],
                    lhsT,
                    x[:, 0:512],
                    start=False,
                    stop=(i == ROWS - 1),
                )
                nc.tensor.matmul(
                    sc_ps[:, 512:1024],
                    lhsT,
                    x[:, 512:1024],
                    start=False,
                    stop=(i == ROWS - 1),
                )
            if mode == "nomm":
                return

            _epilogue(nc, cp, mybir, f32, u8, sc_ps, b2_s, nb2_s, w_out, m_out)


def _epilogue(nc, cp, mybir, f32, u8, sc_ps, b2_s, nb2_s, w_out, m_out):
    # diagonal score entries hold -BIG: sigmoid -> 0 weight, is_gt -> 0 mask
    sig_s = cp.tile([ROWS, N], mybir.dt.bfloat16, tag="sig")
    nc.scalar.activation(
        sig_s[:], sc_ps[:], mybir.ActivationFunctionType.Sigmoid, bias=b2_s[:]
    )
    nc.sync.dma_start(out=w_out[:], in_=sig_s[:])

    m_s = cp.tile([ROWS, N], u8, tag="m")
    nc.vector.tensor_scalar(
        out=m_s[:],
        in0=sc_ps[:],
        scalar1=nb2_s[:],
        scalar2=None,
        op0=mybir.AluOpType.is_gt,
    )
    nc.sync.dma_start(out=m_out[:], in_=m_s[:])


def _build_in_maps(inputs):
    node_emb = np.asarray(inputs["node_emb"], dtype=np.float32)
    W1 = np.asarray(inputs["W1"], dtype=np.float32)
    b1 = np.asarray(inputs["b1"], dtype=np.float32)
    W2 = np.asarray(inputs["W2"], dtype=np.float32)
    b2 = np.asarray(inputs["b2"], dtype=np.float32)

    emb_t = np.ascontiguousarray(node_emb.T)  # [H, N]
    w1a_t = np.ascontiguousarray(W1[:, :H].T)  # [e, h]
    w1b_t = np.ascontiguousarray(W1[:, H:].T)
    b1_col = np.ascontiguousarray(b1.reshape(H, 1))
    zbuf = np.zeros((H, 2 * H), dtype=np.float32)
    zbuf[:, H] = W2[0]
    b2v = np.float32(b2.reshape(-1)[0])
    b2_col = np.full((H, 1), b2v, dtype=np.float32)
    negb2_col = -b2_col

    negbig_eye = np.zeros((H, H), dtype=np.float32)
    np.fill_diagonal(negbig_eye, np.float32(-1e30))

    # v7/v8 off-PE path operands: |W2| prescaled weights, sign broadcast.
    # h axis permuted sign-descending so ACT reduces get contiguous
    # positive/negative groups; _V8_PZ records the split for _build.
    absw2 = np.abs(W2[0]).astype(np.float32)
    sgn = np.sign(W2[0]).astype(np.float32)
    perm = np.argsort(-sgn, kind="stable")
    _V8_PZ[0] = int((sgn > 0).sum())
    _V8_PZ[1] = int((sgn == 0).sum())
    sgn_p = sgn[perm]
    absw2_p = absw2[perm]
    w1aw_t = np.ascontiguousarray(w1a_t[:, perm] * absw2_p[None, :])
    w1bw_t = np.ascontiguousarray(w1b_t[:, perm] * absw2_p[None, :])
    b1w_row = np.ascontiguousarray((b1[perm] * absw2_p).reshape(1, H))
    ones_row = np.ones((1, ROWS), dtype=np.float32)
    sgn_bc = np.ascontiguousarray(np.tile(sgn_p.reshape(1, H), (H, 1)))

    in_maps = []
    for c in range(NCORES):
        r0 = c * ROWS
        in_maps.append(
            {
                "emb_t": emb_t,
                "emb_rows_t": np.ascontiguousarray(emb_t[:, r0 : r0 + ROWS]),
                "w1a_t": w1a_t,
                "w1b_t": w1b_t,
                "b1_col": b1_col,
                "zbuf": zbuf,
                "b2_col": b2_col,
                "negb2_col": negb2_col,
                "rowcol": (r0 + np.arange(ROWS, dtype=np.float32)).reshape(ROWS, 1),
                "negbig_eye": negbig_eye,
                "w1aw_t": w1aw_t,
                "w1bw_t": w1bw_t,
                "b1w_row": b1w_row,
                "ones_row": ones_row,
                "sgn_bc": sgn_bc,
            }
        )
    return in_maps


def _make_runner(nc):
    """Build a reusable jitted runner (mirrors bass2jax.run_bass_via_pjrt,
    but cached so repeated kernel() calls skip re-tracing/compiling)."""
    import jax
    import concourse.mybir as mybir
    from jax.sharding import Mesh, PartitionSpec

    try:
        from jax.experimental.shard_map import shard_map
    except ImportError:
        from jax.shard_map import shard_map

    from concourse.bass2jax import (
        _bass_exec_p,
        install_neuronx_cc_hook,
        partition_id_tensor,
    )

    install_neuronx_cc_hook()
    partition_name = nc.partition_id_tensor.name if nc.partition_id_tensor else None

    in_names, out_names, out_avals, zero_outs = [], [], [], []
    for alloc in nc.m.functions[0].allocations:
        if not isinstance(alloc, mybir.MemoryLocationSet):
            continue
        name = alloc.memorylocations[0].name
        if alloc.kind == "ExternalInput":
            if name != partition_name:
                in_names.append(name)
        elif alloc.kind == "ExternalOutput":
            out_names.append(name)
            shape = tuple(alloc.tensor_shape)
            dtype = mybir.dt.np(alloc.dtype)
            out_avals.append(jax.core.ShapedArray(shape, dtype))
            zero_outs.append(np.zeros(shape, dtype))
    n_params = len(in_names)
    all_in_names = list(in_names) + list(out_names)
    if partition_name is not None:
        all_in_names.append(partition_name)

    def _body(*args):
        operands = list(args)
        if partition_name is not None:
            operands.append(partition_id_tensor())
        return tuple(
            _bass_exec_p.bind(
                *operands,
                out_avals=tuple(out_avals),
                in_names=tuple(all_in_names),
                out_names=tuple(out_names),
                lowering_input_output_aliases=(),
                sim_require_finite=True,
                sim_require_nnan=True,
                nc=nc,
            )
        )

    devices = jax.devices()[:NCORES]
    mesh = Mesh(np.asarray(devices), ("core",))
    n_outs = len(out_avals)
    # only these inputs differ per core; the rest are replicated and ship
    # to the devices once instead of 8 concatenated copies
    per_core_names = {"emb_rows_t", "rowcol"}
    in_specs = tuple(
        PartitionSpec("core") if n in per_core_names else PartitionSpec(None)
        for n in in_names
    ) + (PartitionSpec("core"),) * n_outs
    out_specs = (PartitionSpec("core"),) * n_outs
    fn = jax.jit(
        shard_map(_body, mesh=mesh, in_specs=in_specs, out_specs=out_specs,
                  check_rep=False),
        keep_unused=True,
    )
    concat_zeros = [
        np.zeros((NCORES * z.shape[0], *z.shape[1:]), z.dtype) for z in zero_outs
    ]
    return fn, in_names, out_names, out_avals, concat_zeros, per_core_names


def _run_cached(in_maps):
    import jax

    if "runner" not in _cache:
        _cache["runner"] = _make_runner(_cache["nc"])
    fn, in_names, out_names, out_avals, concat_zeros, per_core_names = _cache["runner"]
    concat_in = [
        np.concatenate([np.asarray(m[name]) for m in in_maps], axis=0)
        if name in per_core_names
        else np.asarray(in_maps[0][name])
        for name in in_names
    ]
    out_arrs = fn(*concat_in, *concat_zeros)
    jax.block_until_ready(out_arrs)
    res = {}
    for i, name in enumerate(out_names):
        res[name] = np.asarray(out_arrs[i]).reshape(
            NCORES, *out_avals[i].shape
        )
    return res


def _postprocess(res):
    """Assemble full outputs from per-core results (either kernel flavor)."""
    if "s_out" in res:
        # v6/v7: res holds bf16 scores+b2; sigmoid/mask/diag on host
        blocks = []
        for c in range(NCORES):
            sc = np.asarray(res["s_out"][c]).astype(np.float32)
            if "soff_out" in res:
                # v7: last K_OFF rows come from the off-PE path:
                # soff[p, 8r+b] = scores[NPE+r, 128b+p]
                soff = np.asarray(res["soff_out"][c]).astype(np.float32)
                ko = soff.shape[-1] // 8
                sc[ROWS - ko :, :] = (
                    soff.reshape(H, ko, 8).transpose(1, 2, 0).reshape(ko, N)
                )
            blocks.append(sc)
        s = np.concatenate(blocks, axis=0)
        weights = 1.0 / (1.0 + np.exp(-s))
        mask = s > 0.0
        np.fill_diagonal(weights, 0.0)
        np.fill_diagonal(mask, False)
        return weights, mask
    weights = np.concatenate(
        [np.asarray(res["w_out"][c]).astype(np.float32) for c in range(NCORES)],
        axis=0,
    )
    mask = np.concatenate(
        [res["m_out"][c] for c in range(NCORES)], axis=0
    ).astype(bool)
    return weights, mask


def kernel(node_emb, W1, b1, W2, b2, temperature=None, **_ignored):
    import time

    if "nc" not in _cache:
        _cache["nc"] = _build()

    in_maps = _build_in_maps(
        {"node_emb": node_emb, "W1": W1, "b1": b1, "W2": W2, "b2": b2}
    )
    # the device occasionally reports NRT_EXEC_UNIT_UNRECOVERABLE if a prior
    # process wedged it; it self-recovers after ~30s, so retry those (and only
    # those) with backoff
    for attempt in range(3):
        try:
            res = _run_cached(in_maps)
            break
        except Exception as e:  # noqa: BLE001
            msg = str(e)
            transient = (
                "UNRECOVERABLE" in msg
                or "unrecoverable" in msg
                or "UNAVAILABLE" in msg
            )
            if attempt == 2 or not transient:
                raise
            time.sleep(30 * (attempt + 1))
    return _postprocess(res)



# revision 48
# speedup vs baseline: 1.0797x; 1.0188x over previous
"""Distributed TRN2 Bass kernel for AdaptiveGraphTopology pairwise edge MLP.

reference:
    a = emb @ W1a.T ; b = emb @ W1b.T           (W1a, W1b = W1[:, :H], W1[:, H:])
    hidden = relu(a[:,None,:] + b[None,:,:] + b1)      # [N,N,H]
    scores = hidden . W2[0] + b2                       # [N,N]
    weights = sigmoid(scores), zeroed diag
    mask    = (weights > 0.5) & ~eye

Sharding: rows i split across 8 cores (128 rows each); everything else
replicated. No collectives: each core DMAs out its row block, host
concatenates.

Shipped mode "v6h" (software-pipelined, fp16 moving data):
    BT[h, j] = b_j[h]        (all j)    -- f32r matmul, copied to fp16
    CT[h, i] = a_i[h]+b1[h]  (local i)  -- f32r matmul + bias
    loop over local i:
      X_i[h, j] = relu(BT[h, j] + CT[h, i])   (DVE 2/3, ACT 1/3; fp16 out)
      scores[i, :] += w2 . X_i  via fp16 matmul whose stationary is a
      sliding window over Z[128, 256] (w2 at column 128, zeros elsewhere):
      window [128-i : 256-i] places w2 in PE column i, so row i's scores
      land in PSUM partition i and the 128 iterations accumulate a full
      [128, 1024] score block (zero columns contribute exact zeros).
    The last 8 rows issue all their half-0 matmuls before the half-1
    burst, so the half-0 bf16-convert + DMA hides under ~2us of PE work.
    Device ships raw bf16 scores(+b2); the host computes sigmoid, mask
    (s > 0: bf16 rounding preserves sign, so the mask is exact for b2=0)
    and zeroes the diagonal — no device sigmoid / is_gt / mask DMA / eye
    machinery at all.

Measured facts (this device, median-of-paired-slope methodology with
(129, 4097) rep contrast — single runs drift +-10 us between process
launches, so only same-run comparisons are valid):
  - The PE streams f32r/bf16 moving data at ~0.54 ns/col with ZERO
    per-matmul overhead (256 vs 512-col MMs, fixed vs sliding
    stationary, bank patterns: all identical). fp16 moving data is
    ~8% faster (~0.49 ns/col) — the only dtype that beats f32r. The
    131072-col score stream is therefore a ~64.5 us floor and the
    whole kernel sits on it (pure-PE probe == full kernel slope).
  - fp16 X costs ~2x the f32r mask flips (146 vs 74 of 1M; gate is
    ~209) because bt, X and the w2 stationary each round to 10-bit
    mantissa; bf16 would blow the flip budget (~290).
  - DVE runs 16-bit tensor_scalar at 2 elem/cycle (all-DVE gens fit in
    ~48 us), but offloading score rows to DVE/ACT/Pool in a [j,h]
    layout LOSES: Pool tensor_tensor is ~10 us per [128,1024] pass,
    ACT accum batches ~10 us/row (pipeline drain per accum_out), DVE
    chains ~2.5 us/row vs the 0.5 us/row the PE pays — engine
    elementwise throughput is ~5-20x too weak to beat the PE stream.
  - tc.For_i puts an all-engine barrier at each iteration end
    (staggered_reset measures ~3 us SLOWER than the barrier).
    BT/CT are rep-invariant, so each body computes them for the NEXT
    rep during its DMA tail, and the prologue seeds the first rep.
"""
import numpy as np

N = 1024
H = 128
NCORES = 8
ROWS = N // NCORES  # 128 rows per core

# v7 tuning: rows whose scores are computed off-PE, and which gen rows go
# to ACT (pattern periods); see _build mode "v7"
K_OFF = 8
ACT_GEN = {1, 3, 5, 8, 10}  # i % 12 in this set -> ACT gen, else DVE

# v8 tuning: XD rows reduced on DVE, XA rows reduced on ACT (sign-split);
# _V8_PZ = (#positive, #zero) sign counts after the host h-permutation,
# set by _build_in_maps before _build runs
V8_XD = 6
V8_XA = 8
_V8_PZ = [64, 0]

_cache = {}


def _split_multiwaits(nc, limit=1):
    """This walrus build accepts only ONE semaphore wait/update per
    instruction; Tile emits several. Split extras onto adjacent NoOps."""
    import bass_rust

    f = nc.m.functions[0]
    engines = nc.engines

    def make_nop(engine_type):
        eng = engines[engine_type]
        inst = eng.nop(nofuse=True).ins
        for b in f.blocks:
            lst = b.instructions
            for k in range(len(lst) - 1, -1, -1):
                if lst[k] is inst:
                    lst.pop(k)
                    return inst
        return inst

    n_split = 0
    for b in f.blocks:
        insts = b.instructions
        i = 0
        while i < len(insts):
            inst = insts[i]
            si = inst.sync_info
            if si is None:
                i += 1
                continue
            waits = list(si.on_wait)
            ups = list(si.on_update)

            def _is_add_imm(u):
                # sem-add-imm consumes the instruction's immediate field; a
                # wait's compare-immediate then conflicts (walrus
                # no_semaphore_value_conflict). sem-inc (+1) needs no imm.
                return (getattr(u, "update_mode", None) == "sem-add-imm"
                        and getattr(u, "update_value", 1) != 1)

            same_sem = (
                len(waits) >= 1 and len(ups) >= 1
                and (any(getattr(w, "id", None) == getattr(u, "id", None)
                         for w in waits for u in ups)
                     or any(_is_add_imm(u) for u in ups))
            )
            if len(waits) <= limit and len(ups) <= 1 and not same_sem:
                i += 1
                continue
            pre = []
            post = []
            if len(waits) > limit:
                extra, waits = waits[: len(waits) - limit], waits[len(waits) - limit :]
                for w in extra:
                    nop = make_nop(inst.engine)
                    nop.sync_info = bass_rust.SyncInfo(on_wait=[w], on_update=[])
                    pre.append(nop)
            if len(ups) > 1:
                ups, extra_u = ups[:1], ups[1:]
                for u in extra_u:
                    nop = make_nop(inst.engine)
                    nop.sync_info = bass_rust.SyncInfo(on_wait=[], on_update=[u])
                    post.append(nop)
            if (waits and ups
                    and (getattr(waits[0], "id", None) == getattr(ups[0], "id", None)
                         or any(_is_add_imm(u) for u in ups))):
                # wait+update on one semaphore — or a wait-imm next to a
                # sem-add-imm update — trips walrus's
                # no_semaphore_value_conflict: hoist the wait onto a
                # preceding NoOp (engine queues are in-order)
                nop = make_nop(inst.engine)
                nop.sync_info = bass_rust.SyncInfo(on_wait=waits, on_update=[])
                pre.append(nop)
                waits = []
            inst.sync_info = bass_rust.SyncInfo(on_wait=waits, on_update=ups)
            insts[i:i] = pre
            i += len(pre)
            if post:
                insts[i + 1 : i + 1] = post
            n_split += 1
            i += 1
    return n_split


def _build(reps=1, loop_reps=1, mode="v6h"):
    import concourse.bass as bass
    import concourse.mybir as mybir
    from concourse.tile import TileContext

    nc = bass.Bass(trn_type="TRN2")
    f32 = mybir.dt.float32
    f32r = mybir.dt.float32r
    u8 = mybir.dt.uint8

    emb_t = nc.dram_tensor("emb_t", [H, N], f32, kind="ExternalInput")
    emb_rows_t = nc.dram_tensor("emb_rows_t", [H, ROWS], f32, kind="ExternalInput")
    w1a_t = nc.dram_tensor("w1a_t", [H, H], f32, kind="ExternalInput")
    w1b_t = nc.dram_tensor("w1b_t", [H, H], f32, kind="ExternalInput")
    b1_col = nc.dram_tensor("b1_col", [H, 1], f32, kind="ExternalInput")
    zbuf = nc.dram_tensor("zbuf", [H, 2 * H], f32, kind="ExternalInput")
    b2_col = nc.dram_tensor("b2_col", [H, 1], f32, kind="ExternalInput")
    negb2_col = nc.dram_tensor("negb2_col", [H, 1], f32, kind="ExternalInput")
    # rowcol[k] = global row index of local row k: used to build the one-hot
    # eye matrix on device (iota + is_equal) that injects -BIG into the
    # diagonal score entries via one accumulating matmul
    rowcol = nc.dram_tensor("rowcol", [ROWS, 1], f32, kind="ExternalInput")
    negbig_eye = nc.dram_tensor("negbig_eye", [H, H], f32, kind="ExternalInput")

    bf16 = mybir.dt.bfloat16
    if mode in ("v7", "v8"):
        # v7 = v6 + row offload: the last K_OFF rows' scores are computed
        # off-PE in [j,h] layout (Pool adds B''+A''bcast, DVE does fused
        # relu*sgn+accum), freeing ~512ns of PE stream per row.
        koff = (V8_XD + V8_XA) if mode == "v8" else K_OFF
        s_out = nc.dram_tensor("s_out", [ROWS, N], bf16, kind="ExternalOutput")
        soff_out = nc.dram_tensor("soff_out", [H, 8 * koff], bf16,
                                  kind="ExternalOutput")
        w_out = m_out = None
        # [j,h]-path host-precomputed operands
        w1aw_t = nc.dram_tensor("w1aw_t", [H, H], f32, kind="ExternalInput")
        w1bw_t = nc.dram_tensor("w1bw_t", [H, H], f32, kind="ExternalInput")
        b1w_row = nc.dram_tensor("b1w_row", [1, H], f32, kind="ExternalInput")
        ones_row = nc.dram_tensor("ones_row", [1, ROWS], f32, kind="ExternalInput")
        sgn_bc = nc.dram_tensor("sgn_bc", [H, H], f32, kind="ExternalInput")
    elif mode.startswith("v6"):
        # v6: device ships raw scores+b2 as bf16; host does sigmoid, mask
        # (sign of bf16 is exact, so mask == f32 mask when b2==0 path is
        # biased on-device) and diagonal zeroing. Kills the device-side
        # sigmoid/is_gt/mask-DMA and the -1e30 eye-init matmuls.
        s_out = nc.dram_tensor("s_out", [ROWS, N], bf16, kind="ExternalOutput")
        w_out = m_out = None
    else:
        # weights leave the core as bf16 (halves the tail DMA); host upcasts.
        # Adds ~1e-3 rel err on weights vs the 2e-2 gate.
        w_out = nc.dram_tensor("w_out", [ROWS, N], bf16, kind="ExternalOutput")
        m_out = nc.dram_tensor("m_out", [ROWS, N], u8, kind="ExternalOutput")

    with TileContext(nc) as tc:
        with (
            tc.tile_pool(name="const", bufs=1) as cp,
            tc.tile_pool(name="xp", bufs=14) as xp,
            tc.tile_pool(name="yp8", bufs=14) as yp8,
            tc.tile_pool(name="pp", bufs=1, space="PSUM") as pp,
        ):
            emba_s = cp.tile([H, 512], f32, tag="emba")
            nc.sync.dma_start(out=emba_s[:], in_=emb_t[:, 0:512])
            embb_s = cp.tile([H, 512], f32, tag="embb")
            nc.sync.dma_start(out=embb_s[:], in_=emb_t[:, 512:1024])
            embr_s = cp.tile([H, ROWS], f32, tag="embr")
            nc.sync.dma_start(out=embr_s[:], in_=emb_rows_t[:])
            w1a_s = cp.tile([H, H], f32, tag="w1a")
            nc.sync.dma_start(out=w1a_s[:], in_=w1a_t[:])
            w1b_s = cp.tile([H, H], f32, tag="w1b")
            nc.sync.dma_start(out=w1b_s[:], in_=w1b_t[:])
            b1_s = cp.tile([H, 1], f32, tag="b1")
            nc.sync.dma_start(out=b1_s[:], in_=b1_col[:])
            z_s = cp.tile([H, 2 * H], f32, tag="z")
            nc.sync.dma_start(out=z_s[:], in_=zbuf[:])
            b2_s = cp.tile([H, 1], f32, tag="b2")
            nc.sync.dma_start(out=b2_s[:], in_=b2_col[:])
            if not mode.startswith(("v6", "v7")):
                nb2_s = cp.tile([H, 1], f32, tag="nb2")
                nc.sync.dma_start(out=nb2_s[:], in_=negb2_col[:])
                rc_s = cp.tile([ROWS, 1], f32, tag="rc")
                nc.sync.dma_start(out=rc_s[:], in_=rowcol[:])
                nbe_s = cp.tile([H, H], f32, tag="nbe")
                nc.sync.dma_start(out=nbe_s[:], in_=negbig_eye[:])

            # round f32r constants once (fp16 stationary for the fp16
            # moving-data variants: matmul can't mix 32/16-bit inputs)
            zdt = mybir.dt.float16 if mode in ("v6h", "v6hd", "v8") else f32r
            zr_s = cp.tile([H, 2 * H], zdt, tag="zr")
            nc.vector.tensor_copy(zr_s[:], z_s[:])
            if not mode.startswith(("v6", "v7")):
                nber_s = cp.tile([H, H], f32r, tag="nber")
                nc.vector.tensor_copy(nber_s[:], nbe_s[:])
            # f32r copies of emb / W1 halves: lets BT/CT run as 1-cycle/col
            # f32r matmuls instead of 4-cycle/col f32 (prologue-only cost)
            embar_s = cp.tile([H, 512], f32r, tag="embar")
            nc.vector.tensor_copy(embar_s[:], emba_s[:])
            embbr_s = cp.tile([H, 512], f32r, tag="embbr")
            nc.vector.tensor_copy(embbr_s[:], embb_s[:])
            embrr_s = cp.tile([H, ROWS], f32r, tag="embrr")
            nc.vector.tensor_copy(embrr_s[:], embr_s[:])
            w1ar_s = cp.tile([H, H], f32r, tag="w1ar")
            nc.vector.tensor_copy(w1ar_s[:], w1a_s[:])
            w1br_s = cp.tile([H, H], f32r, tag="w1br")
            nc.vector.tensor_copy(w1br_s[:], w1b_s[:])

            if not mode.startswith(("v6", "v7")):
                # build the one-hot eye matrix on device: eyr[k, j] = (j == rowcol[k])
                it_s = cp.tile([ROWS, N], f32, tag="it")
                nc.gpsimd.iota(it_s[:], pattern=[[1, N]], base=0,
                               channel_multiplier=0,
                               allow_small_or_imprecise_dtypes=True)
                eyr_s = cp.tile([ROWS, N], f32r, tag="eyr")
                nc.vector.tensor_scalar(
                    out=eyr_s[:],
                    in0=it_s[:],
                    scalar1=rc_s[:],
                    scalar2=None,
                    op0=mybir.AluOpType.is_equal,
                )

            if mode in ("v7", "v8"):
                # [j,h]-path constants
                w1aw_s = cp.tile([H, H], f32, tag="w1aw")
                nc.sync.dma_start(out=w1aw_s[:], in_=w1aw_t[:])
                w1bw_s = cp.tile([H, H], f32, tag="w1bw")
                nc.sync.dma_start(out=w1bw_s[:], in_=w1bw_t[:])
                b1w_s = cp.tile([1, H], f32, tag="b1w")
                nc.sync.dma_start(out=b1w_s[:], in_=b1w_row[:])
                ones_s0 = cp.tile([1, ROWS], f32, tag="ones0")
                nc.sync.dma_start(out=ones_s0[:], in_=ones_row[:])
                sgn_s = cp.tile([H, H], f32, tag="sgn")
                nc.sync.dma_start(out=sgn_s[:], in_=sgn_bc[:])
                w1awr_s = cp.tile([H, H], f32r, tag="w1awr")
                nc.vector.tensor_copy(w1awr_s[:], w1aw_s[:])
                w1bwr_s = cp.tile([H, H], f32r, tag="w1bwr")
                nc.vector.tensor_copy(w1bwr_s[:], w1bw_s[:])
                b1wr_s = cp.tile([1, H], f32r, tag="b1wr")
                nc.vector.tensor_copy(b1wr_s[:], b1w_s[:])
                ones_s = cp.tile([1, ROWS], f32r, tag="ones")
                nc.vector.tensor_copy(ones_s[:], ones_s0[:])

            # warm the PE HAM (clock gate) with dummy f32 matmuls while the
            # large input DMAs land, so prep + early main-loop matmuls run at
            # 2.4 GHz instead of the cold 1.2 GHz
            warm_ps = pp.tile([H, 128], f32, tag="warmp")
            for _w in range(12):
                nc.tensor.matmul(
                    warm_ps[:], w1a_s[:], w1a_s[:], start=True, stop=True
                )

            # force the sigmoid ACT table set to load during prep, so the
            # epilogue sigmoid doesn't pay a ~2.7us mid-kernel table swap
            # (relu/identity are filler entries in every set); reading
            # warm_ps also keeps the warm matmuls alive through DCE.
            # v6 has no device sigmoid: Identity is enough to defeat DCE.
            warm_s = cp.tile([H, 1], f32, tag="warm")
            nc.scalar.activation(
                warm_s[:], warm_ps[:, 0:1],
                mybir.ActivationFunctionType.Identity if mode.startswith(("v6", "v7"))
                else mybir.ActivationFunctionType.Sigmoid
            )

            if mode == "v8":
                # fp16 everywhere + row offload: all gens on DVE (2x mode),
                # XD rows reduced on DVE (relu*sgn + inner-axis reduce),
                # XA rows reduced on ACT (contiguous sign groups via host
                # h-permutation), adds on DVE.
                fp16 = mybir.dt.float16
                K_TAIL = 8
                KV8 = V8_XD + V8_XA
                NPE = ROWS - KV8
                PPOS, ZZ = _V8_PZ[0], _V8_PZ[1]
                NEG0 = PPOS + ZZ  # first negative-sign h index
                ct_ps = pp.tile([H, ROWS], f32, tag="ctp")
                ct_s = cp.tile([H, ROWS], f32, tag="ct")
                bt_ps = pp.tile([H, N], f32, tag="btp")
                bt_s = cp.tile([H, N], fp16, tag="bt")
                sc_ps = pp.tile([ROWS, N], f32, tag="scores")
                sb_s = cp.tile([ROWS, N], bf16, tag="sout")
                ct2_ps = pp.tile([ROWS, H], f32, tag="ct2p")
                ct2_s = cp.tile([ROWS, H], f32r, tag="ct2")
                bq_ps = bt_ps  # prep-only lifetimes: share the 2 PSUM banks
                bq_s = cp.tile([H, 8 * H], fp16, tag="bq")
                # 4 abc slots in one PSUM bank
                abc_ps = pp.tile([ROWS, 4 * H], f32, tag="abcp")
                abc_sb = [cp.tile([ROWS, H], fp16, tag=f"abs{u}",
                                  name=f"abs{u}") for u in range(KV8)]
                acc_s = cp.tile([H, 8 * KV8], f32, tag="acc")
                apn_s = cp.tile([H, 16 * KV8], f32, tag="apn")
                abf_s = cp.tile([H, 8 * KV8], bf16, tag="abf")
                junk_s = cp.tile([H, H], fp16, tag="junk")
                sgnh_s = cp.tile([H, H], fp16, tag="sgnh")
                nc.vector.tensor_copy(sgnh_s[:], sgn_s[:])
                ct2pk = cp.tile([1, KV8 * H], f32r, tag="ct2pk")

                def _rep8(ap):
                    lay = [list(d) for d in ap.ap]
                    return bass.AP(ap.tensor, ap.offset,
                                   [lay[0], [0, 8]] + lay[1:])

                def _blocked(ap):
                    # [128, 1024] viewed as [128, 8, 128] for inner reduce
                    lay = [list(d) for d in ap.ap]
                    return bass.AP(ap.tensor, ap.offset,
                                   [lay[0], [128, 8], [1, 128]])

                def prep():
                    nc.tensor.matmul(ct_ps[:], w1ar_s[:], embrr_s[:],
                                     start=True, stop=True)
                    nc.scalar.activation(
                        ct_s[:], ct_ps[:],
                        mybir.ActivationFunctionType.Identity, bias=b1_s[:])
                    nc.tensor.matmul(bt_ps[:, 0:512], w1br_s[:], embar_s[:],
                                     start=True, stop=True)
                    nc.tensor.matmul(bt_ps[:, 512:1024], w1br_s[:], embbr_s[:],
                                     start=True, stop=True)
                    nc.vector.tensor_copy(bt_s[:, 0:512], bt_ps[:, 0:512])
                    nc.vector.tensor_copy(bt_s[:, 512:1024], bt_ps[:, 512:1024])
                    nc.tensor.matmul(ct2_ps[:], embrr_s[:], w1awr_s[:],
                                     start=True, stop=False)
                    nc.tensor.matmul(ct2_ps[:], ones_s[:], b1wr_s[:],
                                     start=False, stop=True)
                    nc.scalar.activation(ct2_s[:], ct2_ps[:],
                                         mybir.ActivationFunctionType.Identity)
                    nc.sync.dma_start(out=ct2pk[:], in_=ct2_s[NPE:ROWS, :])
                    for bb in range(8):
                        lhsT = (embar_s[:, 128 * bb : 128 * bb + 128] if bb < 4
                                else embbr_s[:, 128 * (bb - 4) : 128 * (bb - 3)])
                        nc.tensor.matmul(
                            bq_ps[:, 128 * bb : 128 * bb + 128], lhsT,
                            w1bwr_s[:], start=True, stop=True)
                    nc.scalar.activation(bq_s[:, 0:512], bq_ps[:, 0:512],
                                         mybir.ActivationFunctionType.Identity)
                    nc.scalar.activation(bq_s[:, 512:1024], bq_ps[:, 512:1024],
                                         mybir.ActivationFunctionType.Identity)

                def abc_build(r):
                    sl = abc_ps[:, H * (r % 4) : H * (r % 4 + 1)]
                    nc.tensor.matmul(sl, ones_s[:],
                                     ct2pk[0:1, H * r : H * (r + 1)],
                                     start=True, stop=True)
                    nc.scalar.activation(abc_sb[r][:], sl,
                                         mybir.ActivationFunctionType.Identity)

                def add_row(r, yp_):
                    y = yp_.tile([H, N], fp16, tag="y", name="y")
                    nc.vector.scalar_tensor_tensor(
                        out=y[:], in0=bq_s[:], scalar=0.0,
                        op0=mybir.AluOpType.add, op1=mybir.AluOpType.add,
                        in1=_rep8(abc_sb[r][:]))
                    return y

                def dve_reduce(r, y):
                    y2 = xp.tile([H, N], fp16, tag="y2")
                    nc.vector.scalar_tensor_tensor(
                        out=y2[:], in0=y[:], scalar=0.0,
                        op0=mybir.AluOpType.max, op1=mybir.AluOpType.mult,
                        in1=_rep8(sgnh_s[:, 0:H]))
                    nc.vector.tensor_reduce(
                        out=acc_s[:, 8 * r : 8 * r + 8], in_=_blocked(y2[:]),
                        axis=mybir.AxisListType.X, op=mybir.AluOpType.add)

                def act_reduce(r, y):
                    for bb in range(8):
                        base = 128 * bb
                        nc.scalar.activation(
                            junk_s[:, 0:PPOS], y[:, base : base + PPOS],
                            mybir.ActivationFunctionType.Relu,
                            accum_out=apn_s[:, 16 * r + bb : 16 * r + bb + 1])
                        nc.scalar.activation(
                            junk_s[:, 0 : H - NEG0], y[:, base + NEG0 : base + H],
                            mybir.ActivationFunctionType.Relu,
                            accum_out=apn_s[:, 16 * r + 8 + bb
                                            : 16 * r + 8 + bb + 1])

                def pn_sub(r):
                    nc.vector.tensor_tensor(
                        out=acc_s[:, 8 * r : 8 * r + 8],
                        in0=apn_s[:, 16 * r : 16 * r + 8],
                        in1=apn_s[:, 16 * r + 8 : 16 * r + 16],
                        op=mybir.AluOpType.subtract)

                def body():
                    pend_h1 = []
                    acts = {}
                    for r in range(KV8):
                        # abc builds front-loaded (one per early row) so the
                        # ACT copies queue ahead of every ACT reduce batch;
                        # pos-neg subs all at the end (they wait on ACT)
                        acts.setdefault(r, []).append(("abc", r))
                        t = 2 + (r * (NPE - 12)) // KV8
                        acts.setdefault(t + 2, []).append(("add", r))
                        acts.setdefault(t + 3, []).append(("red", r))
                    ys = {}
                    for i in range(NPE):
                        x = xp.tile([H, N], fp16, tag="x")
                        nc.vector.tensor_scalar(
                            out=x[:], in0=bt_s[:],
                            scalar1=ct_s[:, i : i + 1], scalar2=0.0,
                            op0=mybir.AluOpType.add,
                            op1=mybir.AluOpType.max)
                        lhsT = zr_s[:, H - i : 2 * H - i]
                        nc.tensor.matmul(
                            sc_ps[:, 0:512], lhsT, x[:, 0:512],
                            start=(i == 0), stop=(i == NPE - 1),
                            skip_group_check=True)
                        if i < NPE - K_TAIL:
                            nc.tensor.matmul(
                                sc_ps[:, 512:1024], lhsT, x[:, 512:1024],
                                start=(i == 0), stop=False,
                                skip_group_check=True)
                        else:
                            pend_h1.append((i, x))
                        for kind, r in acts.get(i, ()):
                            if kind == "abc":
                                abc_build(r)
                            elif kind == "add":
                                ys[r] = add_row(r, yp8)
                            elif kind == "red":
                                if r < V8_XD:
                                    dve_reduce(r, ys[r])
                                else:
                                    act_reduce(r, ys[r])
                    for r in range(V8_XD, KV8):
                        pn_sub(r)
                    nc.scalar.activation(
                        sb_s[:, 0:512], sc_ps[:, 0:512],
                        mybir.ActivationFunctionType.Identity, bias=b2_s[:])
                    nc.sync.dma_start(out=s_out[:, 0:512], in_=sb_s[:, 0:512])
                    for i, x in pend_h1:
                        lhsT = zr_s[:, H - i : 2 * H - i]
                        nc.tensor.matmul(
                            sc_ps[:, 512:1024], lhsT, x[:, 512:1024],
                            start=False, stop=(i == NPE - 1),
                            skip_group_check=True)
                    nc.vector.tensor_scalar(
                        out=sb_s[:, 512:1024], in0=sc_ps[:, 512:1024],
                        scalar1=b2_s[:], scalar2=None,
                        op0=mybir.AluOpType.add)
                    nc.sync.dma_start(out=s_out[:, 512:1024],
                                      in_=sb_s[:, 512:1024])
                    nc.vector.tensor_scalar(
                        out=abf_s[:], in0=acc_s[:], scalar1=b2_s[:],
                        scalar2=None, op0=mybir.AluOpType.add)
                    nc.sync.dma_start(out=soff_out[:], in_=abf_s[:])
                    prep()

                prep()
            elif mode == "v7":
                # v6 structure + off-PE computation of the last K_OFF rows
                K_TAIL = 8
                NPE = ROWS - K_OFF
                ct_ps = pp.tile([H, ROWS], f32, tag="ctp")
                ct_s = cp.tile([H, ROWS], f32, tag="ct")
                bt_ps = pp.tile([H, N], f32, tag="btp")
                bt_s = cp.tile([H, N], f32, tag="bt")
                sc_ps = pp.tile([ROWS, N], f32, tag="scores")
                sb_s = cp.tile([ROWS, N], bf16, tag="sout")
                ct2_ps = pp.tile([ROWS, H], f32, tag="ct2p")
                ct2_s = cp.tile([ROWS, H], f32r, tag="ct2")
                bq_ps = bt_ps  # prep-only lifetimes: share the 2 PSUM banks
                bq_s = cp.tile([H, 8 * H], f32, tag="bq")
                abc_ps = [pp.tile([ROWS, H], f32, tag="abcp", name="abcp")] * 2
                abc_sb = [cp.tile([ROWS, H], f32, tag=f"abcs{u}",
                                  name=f"abcs{u}") for u in range(2)]
                acc_s = cp.tile([H, 8 * K_OFF], f32, tag="acc")
                abf_s = cp.tile([H, 8 * K_OFF], bf16, tag="abf")
                junk_s = cp.tile([H, H], f32, tag="junk")
                # PE moving APs must start at partition 0/32/64: pack the
                # K_OFF ct2 rows into partition 0's free dim via DMA
                ct2pk = cp.tile([1, K_OFF * H], f32r, tag="ct2pk")

                def _rep8(ap):
                    a = ap.copy()
                    a.ap = [a.ap[0], [0, 8]] + a.ap[1:]
                    return a

                def prep():
                    nc.tensor.matmul(ct_ps[:], w1ar_s[:], embrr_s[:],
                                     start=True, stop=True)
                    nc.scalar.activation(
                        ct_s[:], ct_ps[:],
                        mybir.ActivationFunctionType.Identity, bias=b1_s[:])
                    nc.tensor.matmul(bt_ps[:, 0:512], w1br_s[:], embar_s[:],
                                     start=True, stop=True)
                    nc.tensor.matmul(bt_ps[:, 512:1024], w1br_s[:], embbr_s[:],
                                     start=True, stop=True)
                    nc.vector.tensor_copy(bt_s[:, 0:512], bt_ps[:, 0:512])
                    nc.vector.tensor_copy(bt_s[:, 512:1024], bt_ps[:, 512:1024])
                    # ct2T[i, h] = |W2_h|*(a_i[h] + b1[h])
                    nc.tensor.matmul(ct2_ps[:], embrr_s[:], w1awr_s[:],
                                     start=True, stop=False)
                    nc.tensor.matmul(ct2_ps[:], ones_s[:], b1wr_s[:],
                                     start=False, stop=True)
                    nc.scalar.activation(ct2_s[:], ct2_ps[:],
                                         mybir.ActivationFunctionType.Identity)
                    nc.sync.dma_start(out=ct2pk[:], in_=ct2_s[NPE:ROWS, :])
                    # B''[j, h] = |W2_h| * (emb @ W1b.T)[j, h], 8 j-blocks
                    for bb in range(8):
                        lhsT = (embar_s[:, 128 * bb : 128 * bb + 128] if bb < 4
                                else embbr_s[:, 128 * (bb - 4) : 128 * (bb - 3)])
                        nc.tensor.matmul(
                            bq_ps[:, 128 * bb : 128 * bb + 128], lhsT,
                            w1bwr_s[:], start=True, stop=True)
                    nc.scalar.activation(bq_s[:, 0:512], bq_ps[:, 0:512],
                                         mybir.ActivationFunctionType.Identity)
                    nc.scalar.activation(bq_s[:, 512:1024], bq_ps[:, 512:1024],
                                         mybir.ActivationFunctionType.Identity)

                def off_row(r):
                    # scores for local row i = NPE + r, computed off-PE
                    u = r % 2
                    nc.tensor.matmul(abc_ps[u][:], ones_s[:],
                                     ct2pk[0:1, H * r : H * (r + 1)],
                                     start=True, stop=True)
                    nc.scalar.activation(abc_sb[u][:], abc_ps[u][:],
                                         mybir.ActivationFunctionType.Identity)
                    y = xp.tile([H, N], f32, tag="y")
                    nc.gpsimd.tensor_tensor(out=y[:], in0=bq_s[:],
                                            in1=_rep8(abc_sb[u][:]),
                                            op=mybir.AluOpType.add)
                    for bb in range(8):
                        nc.vector.scalar_tensor_tensor(
                            out=junk_s[:], in0=y[:, 128 * bb : 128 * bb + 128],
                            scalar=0.0, op0=mybir.AluOpType.max,
                            op1=mybir.AluOpType.mult, in1=sgn_s[:],
                            accum_out=acc_s[:, 8 * r + bb : 8 * r + bb + 1])

                def body():
                    pend_h1 = []
                    trigger = {max(0, ((r + 1) * NPE) // K_OFF - 4): r
                               for r in range(K_OFF)}
                    for i in range(NPE):
                        x = xp.tile([H, N], f32r, tag="x")
                        if i % 12 in ACT_GEN:
                            nc.scalar.activation(
                                x[:], bt_s[:],
                                mybir.ActivationFunctionType.Relu,
                                bias=ct_s[:, i : i + 1])
                        else:
                            nc.vector.tensor_scalar(
                                out=x[:], in0=bt_s[:],
                                scalar1=ct_s[:, i : i + 1], scalar2=0.0,
                                op0=mybir.AluOpType.add,
                                op1=mybir.AluOpType.max)
                        lhsT = zr_s[:, H - i : 2 * H - i]
                        nc.tensor.matmul(
                            sc_ps[:, 0:512], lhsT, x[:, 0:512],
                            start=(i == 0), stop=(i == NPE - 1),
                            skip_group_check=True)
                        if i < NPE - K_TAIL:
                            nc.tensor.matmul(
                                sc_ps[:, 512:1024], lhsT, x[:, 512:1024],
                                start=(i == 0), stop=False,
                                skip_group_check=True)
                        else:
                            pend_h1.append((i, x))
                        if i in trigger:
                            off_row(trigger[i])
                    nc.scalar.activation(
                        sb_s[:, 0:512], sc_ps[:, 0:512],
                        mybir.ActivationFunctionType.Identity, bias=b2_s[:])
                    nc.sync.dma_start(out=s_out[:, 0:512], in_=sb_s[:, 0:512])
                    for i, x in pend_h1:
                        lhsT = zr_s[:, H - i : 2 * H - i]
                        nc.tensor.matmul(
                            sc_ps[:, 512:1024], lhsT, x[:, 512:1024],
                            start=False, stop=(i == NPE - 1),
                            skip_group_check=True)
                    nc.vector.tensor_scalar(
                        out=sb_s[:, 512:1024], in0=sc_ps[:, 512:1024],
                        scalar1=b2_s[:], scalar2=None,
                        op0=mybir.AluOpType.add)
                    nc.sync.dma_start(out=s_out[:, 512:1024],
                                      in_=sb_s[:, 512:1024])
                    nc.vector.tensor_scalar(
                        out=abf_s[:], in0=acc_s[:], scalar1=b2_s[:],
                        scalar2=None, op0=mybir.AluOpType.add)
                    nc.sync.dma_start(out=soff_out[:], in_=abf_s[:])
                    prep()

                prep()
            elif mode in ("v6", "v6pool", "v6h", "v6hd"):
                # software-pipelined like v4, but: no eye-init (host zeroes
                # the diagonal), raw bf16 scores out (host sigmoid/mask),
                # and the last K_TAIL rows issue all their half-0 matmuls
                # before the half-1 burst so the half-0 convert+DMA hides
                # under ~2us of remaining PE work.
                K_TAIL = 8
                # v6h: bt/X in fp16 — same PE column rate, DVE gens in 2x
                # mode, ~f32r-level precision (10-bit mantissa)
                xdt = mybir.dt.float16 if mode in ("v6h", "v6hd") else f32r
                btdt = mybir.dt.float16 if mode in ("v6h", "v6hd") else f32
                ct_ps = pp.tile([H, ROWS], f32, tag="ctp")
                ct_s = cp.tile([H, ROWS], f32, tag="ct")
                bt_ps = pp.tile([H, N], f32, tag="btp")
                bt_s = cp.tile([H, N], btdt, tag="bt")
                sc_ps = pp.tile([ROWS, N], f32, tag="scores")
                sb_s = cp.tile([ROWS, N], bf16, tag="sout")

                def prep():
                    nc.tensor.matmul(ct_ps[:], w1ar_s[:], embrr_s[:],
                                     start=True, stop=True)
                    nc.scalar.activation(
                        ct_s[:], ct_ps[:],
                        mybir.ActivationFunctionType.Identity, bias=b1_s[:])
                    nc.tensor.matmul(bt_ps[:, 0:512], w1br_s[:], embar_s[:],
                                     start=True, stop=True)
                    nc.tensor.matmul(bt_ps[:, 512:1024], w1br_s[:], embbr_s[:],
                                     start=True, stop=True)
                    nc.vector.tensor_copy(bt_s[:, 0:512], bt_ps[:, 0:512])
                    nc.vector.tensor_copy(bt_s[:, 512:1024], bt_ps[:, 512:1024])

                def _rep_ap(ap, reps):
                    # stride-0 free-dim repeat: [p] + [0, reps] + [inner]
                    a = ap.copy()
                    a.ap = [a.ap[0], [0, reps]] + a.ap[1:]
                    return a

                if mode == "v6pool":
                    # throughput probe: does the idle Pool engine absorb
                    # ~12 rows' worth of [j,h]-style add + relu*sgn+accum
                    # work without moving the critical path?
                    KPROBE = int(__import__("os").environ.get("KPROBE", "8"))
                    ydum = cp.tile([H, N], f32, tag="ydum")
                    acc = cp.tile([H, KPROBE], f32, tag="acc")
                    in0_rep = _rep_ap(emba_s[:, 0:512], 2)
                    in1_rep = _rep_ap(w1a_s[:], 8)

                def pool_probe():
                    for r in range(KPROBE):
                        nc.gpsimd.tensor_tensor(
                            out=ydum[:], in0=in0_rep, in1=in1_rep,
                            op=mybir.AluOpType.add)

                def body():
                    if mode == "v6pool":
                        pool_probe()
                    pend_h1 = []
                    for i in range(ROWS):
                        x = xp.tile([H, N], xdt, tag="x")
                        if i % 3 == 1 and mode != "v6hd":
                            nc.scalar.activation(
                                x[:], bt_s[:],
                                mybir.ActivationFunctionType.Relu,
                                bias=ct_s[:, i : i + 1])
                        else:
                            nc.vector.tensor_scalar(
                                out=x[:], in0=bt_s[:],
                                scalar1=ct_s[:, i : i + 1], scalar2=0.0,
                                op0=mybir.AluOpType.add,
                                op1=mybir.AluOpType.max)
                        lhsT = zr_s[:, H - i : 2 * H - i]
                        nc.tensor.matmul(
                            sc_ps[:, 0:512], lhsT, x[:, 0:512],
                            start=(i == 0), stop=(i == ROWS - 1),
                            skip_group_check=True)
                        if i < ROWS - K_TAIL:
                            nc.tensor.matmul(
                                sc_ps[:, 512:1024], lhsT, x[:, 512:1024],
                                start=(i == 0), stop=False,
                                skip_group_check=True)
                        else:
                            pend_h1.append((i, x))
                    # half-0 finished: convert+DMA it on ACT while the PE
                    # drains the deferred half-1 matmuls
                    nc.scalar.activation(
                        sb_s[:, 0:512], sc_ps[:, 0:512],
                        mybir.ActivationFunctionType.Identity, bias=b2_s[:])
                    nc.sync.dma_start(out=s_out[:, 0:512], in_=sb_s[:, 0:512])
                    for i, x in pend_h1:
                        lhsT = zr_s[:, H - i : 2 * H - i]
                        nc.tensor.matmul(
                            sc_ps[:, 512:1024], lhsT, x[:, 512:1024],
                            start=False, stop=(i == ROWS - 1),
                            skip_group_check=True)
                    # half-1 convert on DVE (ACT may still be busy with h0)
                    nc.vector.tensor_scalar(
                        out=sb_s[:, 512:1024], in0=sc_ps[:, 512:1024],
                        scalar1=b2_s[:], scalar2=None,
                        op0=mybir.AluOpType.add)
                    nc.sync.dma_start(out=s_out[:, 512:1024],
                                      in_=sb_s[:, 512:1024])
                    prep()

                prep()
            elif mode in ("full", "v4", "v4s", "v5", "v5s"):
                # software-pipelined: BT/CT/eye-init are rep-invariant, so
                # each body computes them for the NEXT rep during the DMA
                # tail; gens read the copies produced by the previous rep
                ct_ps = pp.tile([H, ROWS], f32, tag="ctp")
                ct_s = cp.tile([H, ROWS], f32, tag="ct")
                bt_ps = pp.tile([H, N], f32, tag="btp")
                bt_s = cp.tile([H, N], f32, tag="bt")
                sc_ps = pp.tile([ROWS, N], f32, tag="scores")
                sig_s = cp.tile([ROWS, N], mybir.dt.bfloat16, tag="sig")
                m_s = cp.tile([ROWS, N], u8, tag="m")

                def prep():
                    nc.tensor.matmul(ct_ps[:], w1ar_s[:], embrr_s[:],
                                     start=True, stop=True)
                    nc.scalar.activation(
                        ct_s[:], ct_ps[:],
                        mybir.ActivationFunctionType.Identity, bias=b1_s[:])
                    nc.tensor.matmul(bt_ps[:, 0:512], w1br_s[:], embar_s[:],
                                     start=True, stop=True)
                    nc.tensor.matmul(bt_ps[:, 512:1024], w1br_s[:], embbr_s[:],
                                     start=True, stop=True)
                    nc.vector.tensor_copy(bt_s[:, 0:512], bt_ps[:, 0:512])
                    nc.vector.tensor_copy(bt_s[:, 512:1024], bt_ps[:, 512:1024])
                    for h0 in (0, 512):
                        nc.tensor.matmul(
                            sc_ps[:, h0 : h0 + 512], nber_s[:],
                            eyr_s[:, h0 : h0 + 512],
                            start=True, stop=False, skip_group_check=True)

                def body():
                    for i in range(ROWS):
                        x = xp.tile([H, N], f32r, tag="x")
                        if i % 3 == 1:
                            nc.scalar.activation(
                                x[:], bt_s[:],
                                mybir.ActivationFunctionType.Relu,
                                bias=ct_s[:, i : i + 1])
                        else:
                            nc.vector.tensor_scalar(
                                out=x[:], in0=bt_s[:],
                                scalar1=ct_s[:, i : i + 1], scalar2=0.0,
                                op0=mybir.AluOpType.add,
                                op1=mybir.AluOpType.max)
                        lhsT = zr_s[:, H - i : 2 * H - i]
                        nc.tensor.matmul(
                            sc_ps[:, 0:512], lhsT, x[:, 0:512],
                            start=False, stop=(i == ROWS - 1),
                            skip_group_check=True)
                        nc.tensor.matmul(
                            sc_ps[:, 512:1024], lhsT, x[:, 512:1024],
                            start=False, stop=(i == ROWS - 1),
                            skip_group_check=True)
                    if mode not in ("v5", "v5s"):
                        for h0 in (0, 512):
                            nc.scalar.activation(
                                sig_s[:, h0 : h0 + 512], sc_ps[:, h0 : h0 + 512],
                                mybir.ActivationFunctionType.Sigmoid, bias=b2_s[:])
                            nc.sync.dma_start(out=w_out[:, h0 : h0 + 512],
                                              in_=sig_s[:, h0 : h0 + 512])
                        nc.vector.tensor_scalar(
                            out=m_s[:], in0=sc_ps[:], scalar1=nb2_s[:],
                            scalar2=None, op0=mybir.AluOpType.is_gt)
                        nc.sync.dma_start(out=m_out[:], in_=m_s[:])
                        prep()
                        return
                    # v5: engine-queue-aware ordering of tail + next-rep prep:
                    # DVE [gens, btcopies, is_gt], ACT [gens, sigmoids, ct],
                    # PE [MMs, BT, CT, eye] so next-rep inputs land earliest
                    for h0 in (0, 512):
                        nc.scalar.activation(
                            sig_s[:, h0 : h0 + 512], sc_ps[:, h0 : h0 + 512],
                            mybir.ActivationFunctionType.Sigmoid, bias=b2_s[:])
                        nc.sync.dma_start(out=w_out[:, h0 : h0 + 512],
                                          in_=sig_s[:, h0 : h0 + 512])
                    nc.tensor.matmul(bt_ps[:, 0:512], w1br_s[:], embar_s[:],
                                     start=True, stop=True)
                    nc.tensor.matmul(bt_ps[:, 512:1024], w1br_s[:], embbr_s[:],
                                     start=True, stop=True)
                    nc.vector.tensor_copy(bt_s[:, 0:512], bt_ps[:, 0:512])
                    nc.vector.tensor_copy(bt_s[:, 512:1024], bt_ps[:, 512:1024])
                    nc.tensor.matmul(ct_ps[:], w1ar_s[:], embrr_s[:],
                                     start=True, stop=True)
                    nc.vector.tensor_scalar(
                        out=m_s[:], in0=sc_ps[:], scalar1=nb2_s[:],
                        scalar2=None, op0=mybir.AluOpType.is_gt)
                    nc.sync.dma_start(out=m_out[:], in_=m_s[:])
                    nc.scalar.activation(
                        ct_s[:], ct_ps[:],
                        mybir.ActivationFunctionType.Identity, bias=b1_s[:])
                    for h0 in (0, 512):
                        nc.tensor.matmul(
                            sc_ps[:, h0 : h0 + 512], nber_s[:],
                            eyr_s[:, h0 : h0 + 512],
                            start=True, stop=False, skip_group_check=True)

                prep()
            elif mode == "v3":
                def body():
                    _body_v3(nc, tc, cp, xp, pp, mybir, f32, f32r, u8,
                             embar_s, embbr_s, embrr_s, w1ar_s, w1br_s, b1_s,
                             zr_s, b2_s, nb2_s, eyr_s, nber_s, w_out, m_out)
            elif mode in ("v2psum", "v2sb"):
                def body():
                    _body_v2(nc, tc, cp, xp, pp, mybir, f32, f32r, u8,
                             embar_s, embbr_s, embrr_s, w1ar_s, w1br_s, b1_s,
                             zr_s, b2_s, nb2_s, eyr_s, nber_s, w_out, m_out,
                             act_src_psum=(mode == "v2psum"))
            else:
                def body():
                    _body_once(nc, tc, cp, xp, pp, mybir, f32, f32r, u8,
                               (emba_s, embb_s), embr_s, w1a_s, w1b_s, b1_s,
                               zr_s, b2_s, nb2_s, eyr_s, nber_s, w_out, m_out,
                               mode)

            if loop_reps > 1:
                # staggered_reset drops the per-iteration all-engine barrier
                # (rolling per-stage sem resets instead), letting engines flow
                # into the next rep while others drain the tail
                with tc.For_i(0, loop_reps, 1,
                              staggered_reset=(mode in ("v4s", "v5s"))):
                    body()
            else:
                for _rep in range(reps):
                    body()

    _split_multiwaits(nc)
    return nc


def _body_v2(nc, tc, cp, xp, pp, mybir, f32, f32r, u8,
             embar_s, embbr_s, embrr_s, w1ar_s, w1br_s, b1_s, zr_s, b2_s,
             nb2_s, eyr_s, nber_s, w_out, m_out, act_src_psum=True):
    """PE-rate-bound body: f32r BT/CT, ACT gens read bt straight from PSUM,
    DVE gens read an SBUF copy; pipelined epilogue halves."""
    # BT = W1b @ embT (f32r matmuls: 1 cyc/col instead of 4)
    bt_ps = pp.tile([H, N], f32, tag="btp")
    nc.tensor.matmul(bt_ps[:, 0:512], w1br_s[:], embar_s[:], start=True, stop=True)
    nc.tensor.matmul(bt_ps[:, 512:1024], w1br_s[:], embbr_s[:], start=True, stop=True)
    # CT = W1a @ embT_rows + b1
    ct_ps = pp.tile([H, ROWS], f32, tag="ctp")
    nc.tensor.matmul(ct_ps[:], w1ar_s[:], embrr_s[:], start=True, stop=True)
    ct_s = cp.tile([H, ROWS], f32, tag="ct")
    nc.scalar.activation(
        ct_s[:], ct_ps[:], mybir.ActivationFunctionType.Identity, bias=b1_s[:]
    )
    # SBUF copy of bt for the DVE generators (DVE from PSUM would drop to 1x)
    bt_s = cp.tile([H, N], f32, tag="bt")
    nc.vector.tensor_copy(bt_s[:], bt_ps[:])

    # scores PSUM, diagonal pre-initialized to -BIG
    sc_ps = pp.tile([ROWS, N], f32, tag="scores")
    for h0 in (0, 512):
        nc.tensor.matmul(
            sc_ps[:, h0 : h0 + 512], nber_s[:], eyr_s[:, h0 : h0 + 512],
            start=True, stop=False,
        )

    # main loop: ACT tiles early (ACT is ready before the bt SBUF copy lands),
    # then interleave so both engines stay fed; ACT reads bt from PSUM
    # ((N+172)/1.2 vs (N+352)/1.2 from SBUF)
    act_set = {0, 1}
    act_set.update(i for i in range(2, ROWS) if i % 3 == 2)
    for i in range(ROWS):
        x = xp.tile([H, N], f32r, tag="x")
        if i in act_set:
            nc.scalar.activation(
                x[:], bt_ps[:] if act_src_psum else bt_s[:],
                mybir.ActivationFunctionType.Relu,
                bias=ct_s[:, i : i + 1],
            )
        else:
            nc.vector.tensor_scalar(
                out=x[:], in0=bt_s[:],
                scalar1=ct_s[:, i : i + 1], scalar2=0.0,
                op0=mybir.AluOpType.add, op1=mybir.AluOpType.max,
            )
        lhsT = zr_s[:, H - i : 2 * H - i]
        nc.tensor.matmul(
            sc_ps[:, 0:512], lhsT, x[:, 0:512],
            start=False, stop=(i == ROWS - 1),
        )
        nc.tensor.matmul(
            sc_ps[:, 512:1024], lhsT, x[:, 512:1024],
            start=False, stop=(i == ROWS - 1),
        )

    # epilogue: halves so the first w_out DMA overlaps the second sigmoid
    sig_s = cp.tile([ROWS, N], mybir.dt.bfloat16, tag="sig")
    for h0 in (0, 512):
        nc.scalar.activation(
            sig_s[:, h0 : h0 + 512], sc_ps[:, h0 : h0 + 512],
            mybir.ActivationFunctionType.Sigmoid, bias=b2_s[:],
        )
        nc.sync.dma_start(out=w_out[:, h0 : h0 + 512], in_=sig_s[:, h0 : h0 + 512])
    m_s = cp.tile([ROWS, N], u8, tag="m")
    nc.vector.tensor_scalar(
        out=m_s[:], in0=sc_ps[:], scalar1=nb2_s[:], scalar2=None,
        op0=mybir.AluOpType.is_gt,
    )
    nc.sync.dma_start(out=m_out[:], in_=m_s[:])


def _body_v3(nc, tc, cp, xp, pp, mybir, f32, f32r, u8,
             embar_s, embbr_s, embrr_s, w1ar_s, w1br_s, b1_s, zr_s, b2_s,
             nb2_s, eyr_s, nber_s, w_out, m_out):
    """fullold dataflow (SBUF-src gens, split bt copies on DVE+ACT) with:
    CT-first head, f32r BT/CT matmuls, pipelined sigmoid/DMA tail."""
    # CT first so ct_s is ready before the first ACT generation
    ct_ps = pp.tile([H, ROWS], f32, tag="ctp")
    nc.tensor.matmul(ct_ps[:], w1ar_s[:], embrr_s[:], start=True, stop=True)
    ct_s = cp.tile([H, ROWS], f32, tag="ct")
    nc.scalar.activation(
        ct_s[:], ct_ps[:], mybir.ActivationFunctionType.Identity, bias=b1_s[:]
    )
    bt_ps = pp.tile([H, N], f32, tag="btp")
    nc.tensor.matmul(bt_ps[:, 0:512], w1br_s[:], embar_s[:], start=True, stop=True)
    nc.tensor.matmul(bt_ps[:, 512:1024], w1br_s[:], embbr_s[:], start=True, stop=True)
    bt_s = cp.tile([H, N], f32, tag="bt")
    nc.vector.tensor_copy(bt_s[:, 0:512], bt_ps[:, 0:512])
    nc.scalar.copy(bt_s[:, 512:1024], bt_ps[:, 512:1024])

    sc_ps = pp.tile([ROWS, N], f32, tag="scores")
    for h0 in (0, 512):
        nc.tensor.matmul(
            sc_ps[:, h0 : h0 + 512], nber_s[:], eyr_s[:, h0 : h0 + 512],
            start=True, stop=False,
        )

    for i in range(ROWS):
        x = xp.tile([H, N], f32r, tag="x")
        if i % 3 == 1:
            nc.scalar.activation(
                x[:], bt_s[:], mybir.ActivationFunctionType.Relu,
                bias=ct_s[:, i : i + 1],
            )
        else:
            nc.vector.tensor_scalar(
                out=x[:], in0=bt_s[:],
                scalar1=ct_s[:, i : i + 1], scalar2=0.0,
                op0=mybir.AluOpType.add, op1=mybir.AluOpType.max,
            )
        lhsT = zr_s[:, H - i : 2 * H - i]
        nc.tensor.matmul(
            sc_ps[:, 0:512], lhsT, x[:, 0:512],
            start=False, stop=(i == ROWS - 1),
        )
        nc.tensor.matmul(
            sc_ps[:, 512:1024], lhsT, x[:, 512:1024],
            start=False, stop=(i == ROWS - 1),
        )

    sig_s = cp.tile([ROWS, N], mybir.dt.bfloat16, tag="sig")
    for h0 in (0, 512):
        nc.scalar.activation(
            sig_s[:, h0 : h0 + 512], sc_ps[:, h0 : h0 + 512],
            mybir.ActivationFunctionType.Sigmoid, bias=b2_s[:],
        )
        nc.sync.dma_start(out=w_out[:, h0 : h0 + 512], in_=sig_s[:, h0 : h0 + 512])
    m_s = cp.tile([ROWS, N], u8, tag="m")
    nc.vector.tensor_scalar(
        out=m_s[:], in0=sc_ps[:], scalar1=nb2_s[:], scalar2=None,
        op0=mybir.AluOpType.is_gt,
    )
    nc.sync.dma_start(out=m_out[:], in_=m_s[:])


def _body_once(nc, tc, cp, xp, pp, mybir, f32, f32r, u8,
               embt_halves, embr_s, w1a_s, w1b_s, b1_s, zr_s, b2_s, nb2_s,
               eyr_s, nber_s, w_out, m_out, mode="full"):
    emba_s, embb_s = embt_halves
    if mode == "empty":
        return
    if True:
        if True:
            # BT = W1b @ embT  (f32, exact): psum half per matmul; each half
            # depends only on its own emb DMA, and the PSUM->SBUF copies run
            # on different engines so they overlap
            bt_ps = pp.tile([H, N], f32, tag="btp")
            nc.tensor.matmul(
                bt_ps[:, 0:512], w1b_s[:], emba_s[:], start=True, stop=True
            )
            nc.tensor.matmul(
                bt_ps[:, 512:1024], w1b_s[:], embb_s[:], start=True, stop=True
            )
            bt_s = cp.tile([H, N], f32, tag="bt")
            nc.vector.tensor_copy(bt_s[:, 0:512], bt_ps[:, 0:512])
            nc.scalar.copy(bt_s[:, 512:1024], bt_ps[:, 512:1024])

            # CT = W1a @ embT_rows + b1  (f32, exact)
            ct_ps = pp.tile([H, ROWS], f32, tag="ctp")
            nc.tensor.matmul(ct_ps[:], w1a_s[:], embr_s[:], start=True, stop=True)
            ct_s = cp.tile([H, ROWS], f32, tag="ct")
            nc.scalar.activation(
                ct_s[:], ct_ps[:], mybir.ActivationFunctionType.Identity, bias=b1_s[:]
            )

            # main loop: accumulate scores into PSUM [128 rows, 1024 cols]
            sc_ps = pp.tile([ROWS, N], f32, tag="scores")
            # initialize each scores bank with -BIG at the diagonal entries
            # (zeros elsewhere): out[k, j] = -BIG*eye[k, j]; keeps the
            # epilogue off the critical tail
            for h0 in (0, 512):
                nc.tensor.matmul(
                    sc_ps[:, h0 : h0 + 512],
                    nber_s[:],
                    eyr_s[:, h0 : h0 + 512],
                    start=True,
                    stop=False,
                )
            xfix = None
            if mode in ("nogen", "nogen_fixw", "nogen_w32", "nogen_1bank",
                        "nogen_fixw32", "nogen_b8", "nogen_256",
                        "nogen_noacc"):
                xfix = cp.tile([H, N], f32r, tag="xfix")
                nc.vector.tensor_copy(xfix[:, 0:256], zr_s[:])
            if mode == "nogen_noacc":
                # 256 fresh-write (start+stop) MMs into one PSUM region:
                # does dropping the accumulate read-modify-write raise the
                # column rate?
                lhsT = zr_s[:, 0:H]
                for i in range(2 * ROWS):
                    nc.tensor.matmul(
                        sc_ps[:, 0:512], lhsT, xfix[:, 0:512],
                        start=True, stop=True, skip_group_check=True,
                    )
                _epilogue(nc, cp, mybir, f32, u8, sc_ps, b2_s, nb2_s, w_out, m_out)
                return
            if mode == "nogen_256":
                # same total moving cols as nogen, but 512 MMs x 256 cols:
                # separates per-MM fixed overhead from cycle-rate
                lhsT = zr_s[:, 0:H]
                for i in range(2 * ROWS):
                    for c0 in (0, 256):
                        nc.tensor.matmul(
                            sc_ps[:, c0 : c0 + 256], lhsT, xfix[:, c0 : c0 + 256],
                            start=False, stop=(i == 2 * ROWS - 1 and c0 == 256),
                        )
                _epilogue(nc, cp, mybir, f32, u8, sc_ps, b2_s, nb2_s, w_out, m_out)
                return
            if mode == "nogen_bf16":
                # 256 MMs x 512 cols with bf16 moving + bf16 stationary:
                # tests whether the ~250ns/MM is f32r-specific or clock/overhead
                bf16 = mybir.dt.bfloat16
                xbf = cp.tile([H, N], bf16, tag="xbf")
                nc.vector.tensor_copy(xbf[:, 0:256], zr_s[:])
                zbf = cp.tile([H, H], bf16, tag="zbf")
                nc.vector.tensor_copy(zbf[:], zr_s[:, 0:H])
                for i in range(2 * ROWS):
                    nc.tensor.matmul(
                        sc_ps[:, 0:512], zbf[:], xbf[:, 0:512],
                        start=False, stop=(i == 2 * ROWS - 1),
                    )
                _epilogue(nc, cp, mybir, f32, u8, sc_ps, b2_s, nb2_s, w_out, m_out)
                return
            if mode == "nogen_1bank":
                # fixed 128-wide stationary, all MMs -> one PSUM bank
                lhsT = zr_s[:, 0:H]
                for i in range(2 * ROWS):
                    nc.tensor.matmul(
                        sc_ps[:, 0:512], lhsT, xfix[:, 0:512],
                        start=False, stop=(i == 2 * ROWS - 1),
                    )
                _epilogue(nc, cp, mybir, f32, u8, sc_ps, b2_s, nb2_s, w_out, m_out)
                return
            if mode == "nogen_fixw32":
                # fixed 32-wide stationary, all MMs -> one PSUM region
                lhsT = zr_s[:, 128:160]
                for i in range(2 * ROWS):
                    nc.tensor.matmul(
                        sc_ps[0:32, 0:512], lhsT, xfix[:, 0:512],
                        start=False, stop=(i == 2 * ROWS - 1),
                    )
                _epilogue(nc, cp, mybir, f32, u8, sc_ps, b2_s, nb2_s, w_out, m_out)
                return
            if mode == "nogen_b8":
                # sliding stationaries, banks switched every 8 rows
                for i0 in range(0, ROWS, 8):
                    for h0 in (0, 512):
                        for i in range(i0, i0 + 8):
                            lhsT = zr_s[:, H - i : 2 * H - i]
                            nc.tensor.matmul(
                                sc_ps[:, h0 : h0 + 512], lhsT, xfix[:, h0 : h0 + 512],
                                start=False,
                                stop=(i == ROWS - 1),
                            )
                _epilogue(nc, cp, mybir, f32, u8, sc_ps, b2_s, nb2_s, w_out, m_out)
                return
            if mode == "nogen_fixw":
                # PE-only, FIXED stationary: isolates LDWEIGHTS cost vs nogen
                lhsT = zr_s[:, 0:H]
                for i in range(ROWS):
                    nc.tensor.matmul(
                        sc_ps[:, 0:512], lhsT, xfix[:, 0:512],
                        start=False, stop=(i == ROWS - 1),
                    )
                    nc.tensor.matmul(
                        sc_ps[:, 512:1024], lhsT, xfix[:, 512:1024],
                        start=False, stop=(i == ROWS - 1),
                    )
                _epilogue(nc, cp, mybir, f32, u8, sc_ps, b2_s, nb2_s, w_out, m_out)
                return
            if mode == "nogen_w32":
                # PE-only, 32-wide sliding stationaries + tile_position groups
                for g in range(4):
                    for k in range(32):
                        lhsT = zr_s[:, H - k : H + 32 - k]
                        for h0 in (0, 512):
                            nc.tensor.matmul(
                                sc_ps[32 * g : 32 * g + 32, h0 : h0 + 512],
                                lhsT,
                                xfix[:, h0 : h0 + 512],
                                start=False,
                                stop=(k == 31),
                                tile_position=(0, 32 * g),
                            )
                _epilogue(nc, cp, mybir, f32, u8, sc_ps, b2_s, nb2_s, w_out, m_out)
                return
            if mode == "full2":
                # col-group tiled reduction: 32-wide stationaries, 4 strips
                for k in range(32):
                    for g in range(4):
                        i = 32 * g + k
                        x = xp.tile([H, N], f32r, tag="x")
                        if (i * 5) % 13 < 5:
                            nc.scalar.activation(
                                x[:],
                                bt_s[:],
                                mybir.ActivationFunctionType.Relu,
                                bias=ct_s[:, i : i + 1],
                            )
                        else:
                            nc.vector.tensor_scalar(
                                out=x[:],
                                in0=bt_s[:],
                                scalar1=ct_s[:, i : i + 1],
                                scalar2=0.0,
                                op0=mybir.AluOpType.add,
                                op1=mybir.AluOpType.max,
                            )
                        lhsT = zr_s[:, H - k : H + 32 - k]
                        for h0 in (0, 512):
                            nc.tensor.matmul(
                                sc_ps[32 * g : 32 * g + 32, h0 : h0 + 512],
                                lhsT,
                                x[:, h0 : h0 + 512],
                                start=(k == 0),
                                stop=(k == 31),
                                tile_position=(0, 32 * g),
                            )
                _epilogue(nc, cp, mybir, f32, u8, sc_ps, b2_s, nb2_s, w_out, m_out)
                return

            for i in range(ROWS):
                if mode != "nogen":
                    x = xp.tile([H, N], f32r, tag="x")
                    if mode == "actgen" or (mode != "dvegen" and i % 3 == 1):
                        # ACT path: relu(in + bias), ~1147ns
                        nc.scalar.activation(
                            x[:],
                            bt_s[:],
                            mybir.ActivationFunctionType.Relu,
                            bias=ct_s[:, i : i + 1],
                        )
                    else:
                        # DVE path: (in + c_i) then max(.,0), ~720ns
                        nc.vector.tensor_scalar(
                            out=x[:],
                            in0=bt_s[:],
                            scalar1=ct_s[:, i : i + 1],
                            scalar2=0.0,
                            op0=mybir.AluOpType.add,
                            op1=mybir.AluOpType.max,
                        )
                else:
                    x = xfix
                if mode == "nomm":
                    continue
                lhsT = zr_s[:, H - i : 2 * H - i]
                nc.tensor.matmul(
                    sc_ps[:, 0:512],
                    lhsT,
                    x[:, 0:512],
                    start=False,
                    stop=(i == ROWS - 1),
                )
                nc.tensor.matmul(
                    sc_ps[:, 512:1024],
                    lhsT,
                    x[:, 512:1024],
                    start=False,
                    stop=(i == ROWS - 1),
                )
            if mode == "nomm":
                return

            _epilogue(nc, cp, mybir, f32, u8, sc_ps, b2_s, nb2_s, w_out, m_out)


def _epilogue(nc, cp, mybir, f32, u8, sc_ps, b2_s, nb2_s, w_out, m_out):
    # diagonal score entries hold -BIG: sigmoid -> 0 weight, is_gt -> 0 mask
    sig_s = cp.tile([ROWS, N], mybir.dt.bfloat16, tag="sig")
    nc.scalar.activation(
        sig_s[:], sc_ps[:], mybir.ActivationFunctionType.Sigmoid, bias=b2_s[:]
    )
    nc.sync.dma_start(out=w_out[:], in_=sig_s[:])

    m_s = cp.tile([ROWS, N], u8, tag="m")
    nc.vector.tensor_scalar(
        out=m_s[:],
        in0=sc_ps[:],
        scalar1=nb2_s[:],
        scalar2=None,
        op0=mybir.AluOpType.is_gt,
    )
    nc.sync.dma_start(out=m_out[:], in_=m_s[:])


def _build_in_maps(inputs):
    node_emb = np.asarray(inputs["node_emb"], dtype=np.float32)
    W1 = np.asarray(inputs["W1"], dtype=np.float32)
    b1 = np.asarray(inputs["b1"], dtype=np.float32)
    W2 = np.asarray(inputs["W2"], dtype=np.float32)
    b2 = np.asarray(inputs["b2"], dtype=np.float32)

    emb_t = np.ascontiguousarray(node_emb.T)  # [H, N]
    w1a_t = np.ascontiguousarray(W1[:, :H].T)  # [e, h]
    w1b_t = np.ascontiguousarray(W1[:, H:].T)
    b1_col = np.ascontiguousarray(b1.reshape(H, 1))
    zbuf = np.zeros((H, 2 * H), dtype=np.float32)
    zbuf[:, H] = W2[0]
    b2v = np.float32(b2.reshape(-1)[0])
    b2_col = np.full((H, 1), b2v, dtype=np.float32)
    negb2_col = -b2_col

    negbig_eye = np.zeros((H, H), dtype=np.float32)
    np.fill_diagonal(negbig_eye, np.float32(-1e30))

    # v7/v8 off-PE path operands: |W2| prescaled weights, sign broadcast.
    # h axis permuted sign-descending so ACT reduces get contiguous
    # positive/negative groups; _V8_PZ records the split for _build.
    absw2 = np.abs(W2[0]).astype(np.float32)
    sgn = np.sign(W2[0]).astype(np.float32)
    perm = np.argsort(-sgn, kind="stable")
    _V8_PZ[0] = int((sgn > 0).sum())
    _V8_PZ[1] = int((sgn == 0).sum())
    sgn_p = sgn[perm]
    absw2_p = absw2[perm]
    w1aw_t = np.ascontiguousarray(w1a_t[:, perm] * absw2_p[None, :])
    w1bw_t = np.ascontiguousarray(w1b_t[:, perm] * absw2_p[None, :])
    b1w_row = np.ascontiguousarray((b1[perm] * absw2_p).reshape(1, H))
    ones_row = np.ones((1, ROWS), dtype=np.float32)
    sgn_bc = np.ascontiguousarray(np.tile(sgn_p.reshape(1, H), (H, 1)))

    in_maps = []
    for c in range(NCORES):
        r0 = c * ROWS
        in_maps.append(
            {
                "emb_t": emb_t,
                "emb_rows_t": np.ascontiguousarray(emb_t[:, r0 : r0 + ROWS]),
                "w1a_t": w1a_t,
                "w1b_t": w1b_t,
                "b1_col": b1_col,
                "zbuf": zbuf,
                "b2_col": b2_col,
                "negb2_col": negb2_col,
                "rowcol": (r0 + np.arange(ROWS, dtype=np.float32)).reshape(ROWS, 1),
                "negbig_eye": negbig_eye,
                "w1aw_t": w1aw_t,
                "w1bw_t": w1bw_t,
                "b1w_row": b1w_row,
                "ones_row": ones_row,
                "sgn_bc": sgn_bc,
            }
        )
    return in_maps


def _make_runner(nc):
    """Build a reusable jitted runner (mirrors bass2jax.run_bass_via_pjrt,
    but cached so repeated kernel() calls skip re-tracing/compiling)."""
    import jax
    import concourse.mybir as mybir
    from jax.sharding import Mesh, PartitionSpec

    try:
        from jax.experimental.shard_map import shard_map
    except ImportError:
        from jax.shard_map import shard_map

    from concourse.bass2jax import (
        _bass_exec_p,
        install_neuronx_cc_hook,
        partition_id_tensor,
    )

    install_neuronx_cc_hook()
    partition_name = nc.partition_id_tensor.name if nc.partition_id_tensor else None

    in_names, out_names, out_avals, zero_outs = [], [], [], []
    for alloc in nc.m.functions[0].allocations:
        if not isinstance(alloc, mybir.MemoryLocationSet):
            continue
        name = alloc.memorylocations[0].name
        if alloc.kind == "ExternalInput":
            if name != partition_name:
                in_names.append(name)
        elif alloc.kind == "ExternalOutput":
            out_names.append(name)
            shape = tuple(alloc.tensor_shape)
            dtype = mybir.dt.np(alloc.dtype)
            out_avals.append(jax.core.ShapedArray(shape, dtype))
            zero_outs.append(np.zeros(shape, dtype))
    n_params = len(in_names)
    all_in_names = list(in_names) + list(out_names)
    if partition_name is not None:
        all_in_names.append(partition_name)

    def _body(*args):
        operands = list(args)
        if partition_name is not None:
            operands.append(partition_id_tensor())
        return tuple(
            _bass_exec_p.bind(
                *operands,
                out_avals=tuple(out_avals),
                in_names=tuple(all_in_names),
                out_names=tuple(out_names),
                lowering_input_output_aliases=(),
                sim_require_finite=True,
                sim_require_nnan=True,
                nc=nc,
            )
        )

    devices = jax.devices()[:NCORES]
    mesh = Mesh(np.asarray(devices), ("core",))
    n_outs = len(out_avals)
    # only these inputs differ per core; the rest are replicated and ship
    # to the devices once instead of 8 concatenated copies
    per_core_names = {"emb_rows_t", "rowcol"}
    in_specs = tuple(
        PartitionSpec("core") if n in per_core_names else PartitionSpec(None)
        for n in in_names
    ) + (PartitionSpec("core"),) * n_outs
    out_specs = (PartitionSpec("core"),) * n_outs
    fn = jax.jit(
        shard_map(_body, mesh=mesh, in_specs=in_specs, out_specs=out_specs,
                  check_rep=False),
        keep_unused=True,
    )
    concat_zeros = [
        np.zeros((NCORES * z.shape[0], *z.shape[1:]), z.dtype) for z in zero_outs
    ]
    return fn, in_names, out_names, out_avals, concat_zeros, per_core_names


def _run_cached(in_maps):
    import jax

    if "runner" not in _cache:
        _cache["runner"] = _make_runner(_cache["nc"])
    fn, in_names, out_names, out_avals, concat_zeros, per_core_names = _cache["runner"]
    concat_in = [
        np.concatenate([np.asarray(m[name]) for m in in_maps], axis=0)
        if name in per_core_names
        else np.asarray(in_maps[0][name])
        for name in in_names
    ]
    out_arrs = fn(*concat_in, *concat_zeros)
    jax.block_until_ready(out_arrs)
    res = {}
    for i, name in enumerate(out_names):
        res[name] = np.asarray(out_arrs[i]).reshape(
            NCORES, *out_avals[i].shape
        )
    return res


def _postprocess(res):
    """Assemble full outputs from per-core results (either kernel flavor)."""
    if "s_out" in res:
        # v6/v7: res holds bf16 scores+b2; sigmoid/mask/diag on host
        blocks = []
        for c in range(NCORES):
            sc = np.asarray(res["s_out"][c]).astype(np.float32)
            if "soff_out" in res:
                # v7: last K_OFF rows come from the off-PE path:
                # soff[p, 8r+b] = scores[NPE+r, 128b+p]
                soff = np.asarray(res["soff_out"][c]).astype(np.float32)
                ko = soff.shape[-1] // 8
                sc[ROWS - ko :, :] = (
                    soff.reshape(H, ko, 8).transpose(1, 2, 0).reshape(ko, N)
                )
            blocks.append(sc)
        s = np.concatenate(blocks, axis=0)
        weights = 1.0 / (1.0 + np.exp(-s))
        mask = s > 0.0
        np.fill_diagonal(weights, 0.0)
        np.fill_diagonal(mask, False)
        return weights, mask
    weights = np.concatenate(
        [np.asarray(res["w_out"][c]).astype(np.float32) for c in range(NCORES)],
        axis=0,
    )
    mask = np.concatenate(
        [res["m_out"][c] for c in range(NCORES)], axis=0
    ).astype(bool)
    return weights, mask


def _spot_check(weights, node_emb, W1, b1, W2, b2, n=512, tol=0.05):
    """Exact recompute of n random pairs: catches silent per-core flakes."""
    rng = np.random.default_rng(0)
    ii = rng.integers(0, N, n)
    jj = rng.integers(0, N, n)
    W1a, W1b = W1[:, :H], W1[:, H:]
    a = node_emb[ii] @ W1a.T
    b = node_emb[jj] @ W1b.T
    hid = np.maximum(a + b + b1[None, :], 0.0)
    sc = hid @ W2[0] + b2.reshape(-1)[0]
    w = 1.0 / (1.0 + np.exp(-sc))
    w = np.where(ii == jj, 0.0, w)
    return float(np.abs(weights[ii, jj] - w).max()) < tol


def kernel(node_emb, W1, b1, W2, b2, temperature=None, **_ignored):
    import time

    in_maps = _build_in_maps(
        {"node_emb": node_emb, "W1": W1, "b1": b1, "W2": W2, "b2": b2}
    )
    if "nc" not in _cache:
        _cache["nc"] = _build()

    # the device occasionally reports NRT_EXEC_UNIT_UNRECOVERABLE if a prior
    # process wedged it (self-recovers after ~30s), and very rarely returns
    # silently corrupted output for one core: retry transient errors with
    # backoff and spot-check the output against an exact recompute of 512
    # random pairs
    out = None
    for attempt in range(3):
        try:
            res = _run_cached(in_maps)
        except Exception as e:  # noqa: BLE001
            msg = str(e)
            transient = (
                "UNRECOVERABLE" in msg
                or "unrecoverable" in msg
                or "UNAVAILABLE" in msg
            )
            if attempt == 2 or not transient:
                raise
            time.sleep(30 * (attempt + 1))
            continue
        out = _postprocess(res)
        if _spot_check(out[0], np.asarray(node_emb, np.float32),
                       np.asarray(W1, np.float32), np.asarray(b1, np.float32),
                       np.asarray(W2, np.float32), np.asarray(b2, np.float32)):
            break
    return out

